# revision 13
# baseline (speedup 1.0000x reference)
"""MixerBlock TRN2 kernel: B=2, S=4096, E=1024, DF=4096 on 8 NeuronCores.

Strategy (two SPMD launches):
  Phase 1 (shard B*S=8192 rows -> 1024 rows/core):
    h   = LN(x)            (cn affine folded into W1/b1 host-side)
    a   = silu(h @ W1g + b1')        -> kept transposed aT[df, tok]
    y   = x + aT.T @ W2 + b2
    h2  = LN(y)*tn_g + tn_b          (bf16)
    outputs y (f32), h2 (bf16)
  Phase 2 (shard E=1024 -> 128 channels/core; rows (b,e) = 256/core):
    out[be, s] = sum_t h2T[t, be] * M[t, s] + tb[s] + y[be, s]
    The Toeplitz matrix M[t,s] = tw[s-t] (s>=t) is diagonal-constant, so a
    [128t x 512s] tile depends only on (512*sb - 128*t): 32 distinct tiles,
    prebuilt host-side from tw (4 MB bf16), used as the moving operand.
"""

import os
import sys

sys.path.insert(0, "/opt/trn_rl_repo")
sys.path.insert(0, "/opt/trn_rl_repo/concourse")

import numpy as np
import ml_dtypes

import concourse.bass as bass
import concourse.bacc as bacc
import concourse.mybir as mybir
from concourse import tile
from concourse import bass_utils
from concourse.bass_interp import get_hw_module

dt = mybir.dt
AF = mybir.ActivationFunctionType
AX = mybir.AxisListType
BF16 = ml_dtypes.bfloat16

B, S, E = 2, 4096, 1024
DF = 4 * E
EPS = 1e-5
NCORES = 8
RPC = (B * S) // NCORES      # 1024 rows per core (phase 1)
EPC = E // NCORES            # 128 channels per core (phase 2)
BE = B * EPC                 # 256 (b,e) rows per core (phase 2)

LAST_TIMINGS = {}

# --------------------------------------------------------------------------
# phase 1 program
# --------------------------------------------------------------------------


def build_phase1():
    nc = bacc.Bacc("TRN2", target_bir_lowering=False, debug=False,
                   enable_asserts=False, num_devices=NCORES)
    x_d = nc.dram_tensor("x", [RPC, E], dt.float32, kind="ExternalInput").ap()
    w1_d = nc.dram_tensor("w1", [E, DF], dt.bfloat16, kind="ExternalInput").ap()
    w2_d = nc.dram_tensor("w2", [DF, E], dt.bfloat16, kind="ExternalInput").ap()
    b1_d = nc.dram_tensor("b1", [128, 32], dt.float32, kind="ExternalInput").ap()
    b2_d = nc.dram_tensor("b2", [128, E], dt.float32, kind="ExternalInput").ap()
    tng_d = nc.dram_tensor("tng", [128, E], dt.bfloat16, kind="ExternalInput").ap()
    tnb_d = nc.dram_tensor("tnb", [128, E], dt.bfloat16, kind="ExternalInput").ap()
    id_d = nc.dram_tensor("ident", [128, 128], dt.bfloat16, kind="ExternalInput").ap()
    y_d = nc.dram_tensor("y", [RPC, E], dt.float32, kind="ExternalOutput").ap()
    h2_d = nc.dram_tensor("h2", [RPC, E], dt.bfloat16, kind="ExternalOutput").ap()

    NT = 4          # token tiles per block (block = 512 tokens)
    NBLK = RPC // (128 * NT)   # 2 blocks

    from contextlib import ExitStack
    with tile.TileContext(nc) as tc, ExitStack() as es:
        pool = lambda **kw: es.enter_context(tc.tile_pool(**kw))
        constp = pool(name="const", bufs=1)
        w1p = pool(name="w1p", bufs=8)
        xp = pool(name="xp", bufs=3)
        xrp = pool(name="xrp", bufs=4)
        xcp = pool(name="xcp", bufs=2)
        sqp = pool(name="sqp", bufs=2)
        statp = pool(name="stat", bufs=24)
        hbfp = pool(name="hbf", bufs=2)
        htp = pool(name="htp", bufs=17)
        atp = pool(name="atp", bufs=34)
        w2p = pool(name="w2p", bufs=4)
        yp = pool(name="yp", bufs=5)
        h2p = pool(name="h2p", bufs=2)
        tpsum = pool(name="tps", bufs=2, space="PSUM")
        m1psum = pool(name="m1ps", bufs=2, space="PSUM")
        m2psum = pool(name="m2ps", bufs=4, space="PSUM")
        if True:
            # small consts first so PE-side deps clear fast
            id_sb = constp.tile([128, 128], dt.bfloat16, tag="ident")
            nc.sync.dma_start(out=id_sb[:, :], in_=id_d[:, :])
            eps_sb = constp.tile([128, 1], dt.float32, tag="eps")
            nc.gpsimd.memset(eps_sb[:, :], EPS)

            def layernorm_to(src, dst_bf, scale_rows=None, bias_rows=None):
                """dst_bf (bf16) = LN(src) [* scale_rows + bias_rows]."""
                ssum = statp.tile([128, 1], dt.float32, tag="ssum")
                nc.vector.reduce_sum(ssum[:, :], src[:, :], axis=AX.X)
                negmean = statp.tile([128, 1], dt.float32, tag="negmean")
                nc.scalar.mul(negmean[:, :], ssum[:, :], -1.0 / E)
                xc = xcp.tile([128, E], dt.float32, tag="xc")
                nc.vector.tensor_scalar_add(xc[:, :], src[:, :], negmean[:, :])
                sq = sqp.tile([128, E], dt.bfloat16, tag="sq")
                ssq = statp.tile([128, 1], dt.float32, tag="ssq")
                nc.scalar.activation(sq[:, :], xc[:, :], AF.Square,
                                     accum_out=ssq[:, :])
                std = statp.tile([128, 1], dt.float32, tag="std")
                nc.scalar.activation(std[:, :], ssq[:, :], AF.Sqrt,
                                     scale=1.0 / E, bias=eps_sb[:, :])
                rstd = statp.tile([128, 1], dt.float32, tag="rstd")
                nc.vector.reciprocal(rstd[:, :], std[:, :])
                nc.scalar.activation(dst_bf[:, :], xc[:, :], AF.Copy,
                                     scale=rstd[:, :])
                if scale_rows is not None:
                    nc.vector.tensor_mul(dst_bf[:, :], dst_bf[:, :],
                                         scale_rows[:, :])
                if bias_rows is not None:
                    nc.vector.tensor_add(dst_bf[:, :], dst_bf[:, :],
                                         bias_rows[:, :])

            # ---- LN1 + transpose for ALL tokens up front ----
            # hT[blk][e] : [e 128, tok 512] bf16
            hT = [[None] * 8 for _ in range(NBLK)]
            for blk in range(NBLK):
                row0 = blk * 128 * NT
                for tt in range(NT):
                    xt = xp.tile([128, E], dt.float32, tag="xt")
                    nc.sync.dma_start(
                        out=xt[:, :],
                        in_=x_d[row0 + tt * 128: row0 + (tt + 1) * 128, :])
                    hb = hbfp.tile([128, E], dt.bfloat16, tag="hb")
                    layernorm_to(xt, hb)
                    for e in range(8):
                        pt = tpsum.tile([128, 128], dt.bfloat16, tag="tp")
                        nc.tensor.transpose(
                            pt[:, :], hb[:, e * 128:(e + 1) * 128], id_sb[:, :])
                        if hT[blk][e] is None:
                            hT[blk][e] = htp.tile([128, 512], dt.bfloat16,
                                                  tag="ht", name=f"ht{blk}_{e}")
                        nc.scalar.copy(
                            hT[blk][e][:, tt * 128:(tt + 1) * 128], pt[:, :])

            # ---- weights (after x in DMA program order) ----
            w1_sb = []
            for i in range(8):
                t = w1p.tile([128, DF], dt.bfloat16, tag="w1sb")
                nc.sync.dma_start(out=t[:, :], in_=w1_d[i * 128:(i + 1) * 128, :])
                w1_sb.append(t)
            b1_sb = constp.tile([128, 32], dt.float32, tag="b1")
            nc.sync.dma_start(out=b1_sb[:, :], in_=b1_d[:, :])
            b2_sb = constp.tile([128, E], dt.float32, tag="b2")
            nc.sync.dma_start(out=b2_sb[:, :], in_=b2_d[:, :])
            tng_sb = constp.tile([128, E], dt.bfloat16, tag="tng")
            nc.sync.dma_start(out=tng_sb[:, :], in_=tng_d[:, :])
            tnb_sb = constp.tile([128, E], dt.bfloat16, tag="tnb")
            nc.sync.dma_start(out=tnb_sb[:, :], in_=tnb_d[:, :])

            for blk in range(NBLK):
                row0 = blk * 128 * NT
                # ---- mm1 + silu -> aT[df][df 128, tok 512] (bf16) ----
                aT = []
                for df in range(32):
                    ps = m1psum.tile([128, 512], dt.float32, tag="m1")
                    for e in range(8):
                        nc.tensor.matmul(
                            ps[:, :],
                            w1_sb[e][:, df * 128:(df + 1) * 128],
                            hT[blk][e][:, :],
                            start=(e == 0), stop=(e == 7))
                    at = atp.tile([128, 512], dt.bfloat16, tag="at")
                    nc.scalar.activation(at[:, :], ps[:, :], AF.Silu,
                                         bias=b1_sb[:, df:df + 1])
                    aT.append(at)
                # ---- mm2 (stream W2) -> y = x + out + b2 ----
                y_t = [yp.tile([128, E], dt.float32, tag="yt",
                               name=f"yt{blk}_{i}") for i in range(NT)]
                for eb in range(2):
                    pss = [m2psum.tile([128, 512], dt.float32, tag="m2",
                                       name=f"m2_{blk}_{eb}_{i}")
                           for i in range(NT)]
                    for df in range(32):
                        w2t = w2p.tile([128, 512], dt.bfloat16, tag="w2t")
                        nc.sync.dma_start(
                            out=w2t[:, :],
                            in_=w2_d[df * 128:(df + 1) * 128,
                                     eb * 512:(eb + 1) * 512])
                        for tt in range(NT):
                            nc.tensor.matmul(
                                pss[tt][:, :],
                                aT[df][:, tt * 128:(tt + 1) * 128],
                                w2t[:, :],
                                start=(df == 0), stop=(df == 31))
                    for tt in range(NT):
                        xr = xrp.tile([128, 512], dt.float32, tag="xr")
                        nc.sync.dma_start(
                            out=xr[:, :],
                            in_=x_d[row0 + tt * 128: row0 + (tt + 1) * 128,
                                    eb * 512:(eb + 1) * 512])
                        ysl = y_t[tt][:, eb * 512:(eb + 1) * 512]
                        nc.vector.tensor_add(ysl, pss[tt][:, :], xr[:, :])
                        nc.gpsimd.tensor_add(
                            ysl, ysl, b2_sb[:, eb * 512:(eb + 1) * 512])
                # ---- write y, LN2 -> h2 ----
                for tt in range(NT):
                    nc.sync.dma_start(
                        out=y_d[row0 + tt * 128: row0 + (tt + 1) * 128, :],
                        in_=y_t[tt][:, :])
                    h2t = h2p.tile([128, E], dt.bfloat16, tag="h2t")
                    layernorm_to(y_t[tt], h2t, scale_rows=tng_sb,
                                 bias_rows=tnb_sb)
                    nc.sync.dma_start(
                        out=h2_d[row0 + tt * 128: row0 + (tt + 1) * 128, :],
                        in_=h2t[:, :])
    nc.compile()
    nc.m = get_hw_module(nc.m)
    return nc


# --------------------------------------------------------------------------
# phase 2 program
# --------------------------------------------------------------------------


def build_phase2():
    nc = bacc.Bacc("TRN2", target_bir_lowering=False, debug=False,
                   enable_asserts=False, num_devices=NCORES)
    h2t_d = nc.dram_tensor("h2t", [S, BE], dt.bfloat16, kind="ExternalInput").ap()
    r_d = nc.dram_tensor("rt", [S, 512], dt.bfloat16, kind="ExternalInput").ap()
    yt_d = nc.dram_tensor("yt", [BE, S], dt.float32, kind="ExternalInput").ap()
    tb_d = nc.dram_tensor("tb", [1, S], dt.bfloat16, kind="ExternalInput").ap()
    ones_d = nc.dram_tensor("ones", [1, 128], dt.bfloat16, kind="ExternalInput").ap()
    out_d = nc.dram_tensor("out", [BE, S], dt.float32, kind="ExternalOutput").ap()

    from contextlib import ExitStack
    with tile.TileContext(nc) as tc, ExitStack() as es:
        pool = lambda **kw: es.enter_context(tc.tile_pool(**kw))
        hsp = pool(name="hs", bufs=32)
        rtp = pool(name="rt", bufs=32)
        constp = pool(name="const", bufs=1)
        yinp = pool(name="yin", bufs=6)
        outp = pool(name="outp", bufs=6)
        psp = pool(name="ps", bufs=8, space="PSUM")
        if True:
            tb_sb = constp.tile([1, S], dt.bfloat16, tag="tb")
            nc.sync.dma_start(out=tb_sb[:, :], in_=tb_d[:, :])
            ones_sb = constp.tile([1, 128], dt.bfloat16, tag="ones")
            nc.sync.dma_start(out=ones_sb[:, :], in_=ones_d[:, :])
            # interleave loads in the order the sb-outer loop consumes them:
            # group g supplies rt[4g..4g+3] and hs[4g..4g+3]
            hs, rt = [None] * 32, [None] * 32
            for g in range(8):
                for k in range(4):
                    d = 4 * g + k
                    rt[d] = rtp.tile([128, 512], dt.bfloat16, tag="rt",
                                     name=f"rt{d}")
                    nc.sync.dma_start(out=rt[d][:, :],
                                      in_=r_d[d * 128:(d + 1) * 128, :])
                for k in range(4):
                    t = 4 * g + k
                    hs[t] = hsp.tile([128, BE], dt.bfloat16, tag="hs",
                                     name=f"hs{t}")
                    nc.sync.dma_start(out=hs[t][:, :],
                                      in_=h2t_d[t * 128:(t + 1) * 128, :])

            for sb in range(8):
                for be in range(2):
                    ps = psp.tile([128, 512], dt.float32, tag="ps",
                                  name=f"ps{sb}_{be}")
                    lhs_col = slice(be * 128, (be + 1) * 128)
                    for t in range(4 * sb + 4):
                        d = 4 * sb - t + 3
                        nc.tensor.matmul(
                            ps[:, :], hs[t][:, lhs_col], rt[d][:, :],
                            start=(t == 0), stop=False)
                    nc.tensor.matmul(
                        ps[:, :], ones_sb[:, :],
                        tb_sb[:, sb * 512:(sb + 1) * 512],
                        start=False, stop=True)
                    yin = yinp.tile([128, 512], dt.float32, tag="yin")
                    nc.sync.dma_start(
                        out=yin[:, :],
                        in_=yt_d[be * 128:(be + 1) * 128,
                                 sb * 512:(sb + 1) * 512])
                    ot = outp.tile([128, 512], dt.float32, tag="ot")
                    nc.vector.tensor_add(ot[:, :], ps[:, :], yin[:, :])
                    nc.sync.dma_start(
                        out=out_d[be * 128:(be + 1) * 128,
                                  sb * 512:(sb + 1) * 512],
                        in_=ot[:, :])
    nc.compile()
    nc.m = get_hw_module(nc.m)
    return nc


def _install_ntff_hook():
    """The agent image's antenv lacks axon_hooks; synthesize it so
    run_bass_kernel_spmd(trace=True) can capture NTFF profiles."""
    import types
    import antenv

    if "antenv.axon_hooks" in sys.modules:
        return
    mod = types.ModuleType("antenv.axon_hooks")
    state = {"h": None}
    mod.set_axon_ntff_profile_hook = lambda h: state.__setitem__("h", h)
    mod.get_axon_ntff_profile_hook = lambda: state["h"]
    sys.modules["antenv.axon_hooks"] = mod
    antenv.axon_hooks = mod
    from trn_agent_boot.trn_boot import _ntff_profile_via_ctypes

    mod.set_axon_ntff_profile_hook(
        _ntff_profile_via_ctypes("/opt/axon/libaxon_pjrt.so"))
    bass_utils.upload_artifacts = lambda tmpdir: tmpdir


_P1 = None
_P2 = None


def _programs():
    global _P1, _P2
    if _P1 is None:
        _P1 = build_phase1()
    if _P2 is None:
        _P2 = build_phase2()
    return _P1, _P2


def _run(nc, in_maps, trace):
    if trace:
        try:
            _install_ntff_hook()
        except Exception as e:
            print(f"ntff hook install failed: {e}", file=sys.stderr)
            trace = False
    res = bass_utils.run_bass_kernel_spmd(
        nc, in_maps, core_ids=list(range(NCORES)), trace=trace)
    return res


def kernel(x, cn_g, cn_b, W1, b1, W2, b2, tn_g, tn_b, tw, tb):
    trace = os.environ.get("MIXER_TRACE", "0") == "1"
    x = np.asarray(x, np.float32)
    p1, p2 = _programs()

    # ---- host prep (inputs only) ----
    W1 = np.asarray(W1, np.float32)
    W2 = np.asarray(W2, np.float32)
    cn_g = np.asarray(cn_g, np.float32)
    cn_b = np.asarray(cn_b, np.float32)
    w1g = (cn_g[:, None] * W1).astype(BF16)
    b1f = (np.asarray(b1, np.float32) + cn_b @ W1).astype(np.float32)
    b1_t = np.ascontiguousarray(b1f.reshape(32, 128).T)          # [128, 32]
    w2bf = W2.astype(BF16)
    b2b = np.ascontiguousarray(
        np.broadcast_to(np.asarray(b2, np.float32), (128, E)))
    tngb = np.ascontiguousarray(
        np.broadcast_to(np.asarray(tn_g, np.float32).astype(BF16), (128, E)))
    tnbb = np.ascontiguousarray(
        np.broadcast_to(np.asarray(tn_b, np.float32).astype(BF16), (128, E)))
    ident = np.eye(128, dtype=BF16)

    xf = x.reshape(B * S, E)
    in_maps1 = []
    for c in range(NCORES):
        in_maps1.append({
            "x": np.ascontiguousarray(xf[c * RPC:(c + 1) * RPC]),
            "w1": w1g, "w2": w2bf, "b1": b1_t, "b2": b2b,
            "tng": tngb, "tnb": tnbb, "ident": ident,
        })
    r1 = _run(p1, in_maps1, trace)
    if trace:
        LAST_TIMINGS["phase1_ns"] = r1.exec_time_ns
    y = np.concatenate([np.asarray(r1.results[c]["y"], np.float32)
                        for c in range(NCORES)], axis=0)
    h2 = np.concatenate([np.asarray(r1.results[c]["h2"]).view(BF16)
                         if r1.results[c]["h2"].dtype != BF16
                         else r1.results[c]["h2"]
                         for c in range(NCORES)], axis=0)

    # ---- phase 2 host glue ----
    tw = np.asarray(tw, np.float32)
    pad = np.zeros(512 + S + 512, np.float32)
    pad[512:512 + S] = tw
    # R[d][i, j] = tw_ext[(d-3)*128 + j - i]
    win = np.lib.stride_tricks.sliding_window_view(pad, 512)   # win[k] = pad[k:k+512]
    rtiles = np.empty((32, 128, 512), np.float32)
    ii = np.arange(128)
    for d in range(32):
        rtiles[d] = win[512 + (d - 3) * 128 - ii]
    rtiles_bf = rtiles.astype(BF16).reshape(S, 512)
    tb_row = np.asarray(tb, np.float32).astype(BF16).reshape(1, S)
    ones_row = np.ones((1, 128), BF16)

    h2v = h2.reshape(B, S, E)
    yv = y.reshape(B, S, E)
    in_maps2 = []
    for c in range(NCORES):
        e0 = c * EPC
        h2sl = np.ascontiguousarray(
            h2v[:, :, e0:e0 + EPC].transpose(1, 0, 2).reshape(S, BE))
        ysl = np.ascontiguousarray(
            yv[:, :, e0:e0 + EPC].transpose(0, 2, 1).reshape(BE, S))
        in_maps2.append({"h2t": h2sl, "rt": rtiles_bf, "yt": ysl,
                         "tb": tb_row, "ones": ones_row})
    r2 = _run(p2, in_maps2, trace)
    if trace:
        LAST_TIMINGS["phase2_ns"] = r2.exec_time_ns

    out = np.empty((B, S, E), np.float32)
    for c in range(NCORES):
        e0 = c * EPC
        o = np.asarray(r2.results[c]["out"], np.float32).reshape(B, EPC, S)
        out[:, :, e0:e0 + EPC] = o.transpose(0, 2, 1)
    return out


# revision 15
# speedup vs baseline: 1.1137x; 1.1137x over previous
"""MixerBlock TRN2 kernel: B=2, S=4096, E=1024, DF=4096 on 8 NeuronCores.

Strategy (two SPMD launches):
  Phase 1 (shard B*S=8192 rows -> 1024 rows/core):
    h   = LN(x)            (cn affine folded into W1/b1 host-side)
    a   = silu(h @ W1g + b1')        -> kept transposed aT[df, tok]
    y   = x + aT.T @ W2 + b2
    h2  = LN(y)*tn_g + tn_b          (bf16)
    outputs y (f32), h2 (bf16)
  Phase 2 (shard E=1024 -> 128 channels/core; rows (b,e) = 256/core):
    out[be, s] = sum_t h2T[t, be] * M[t, s] + tb[s] + y[be, s]
    The Toeplitz matrix M[t,s] = tw[s-t] (s>=t) is diagonal-constant, so a
    [128t x 512s] tile depends only on (512*sb - 128*t): 32 distinct tiles,
    prebuilt host-side from tw (4 MB bf16), used as the moving operand.
"""

import os
import sys

sys.path.insert(0, "/opt/trn_rl_repo")
sys.path.insert(0, "/opt/trn_rl_repo/concourse")

import numpy as np
import ml_dtypes

import concourse.bass as bass
import concourse.bacc as bacc
import concourse.mybir as mybir
from concourse import tile
from concourse import bass_utils
from concourse.bass_interp import get_hw_module

dt = mybir.dt
AF = mybir.ActivationFunctionType
AX = mybir.AxisListType
BF16 = ml_dtypes.bfloat16

B, S, E = 2, 4096, 1024
DF = 4 * E
EPS = 1e-5
NCORES = 8
RPC = (B * S) // NCORES      # 1024 rows per core (phase 1)
EPC = E // NCORES            # 128 channels per core (phase 2)
BE = B * EPC                 # 256 (b,e) rows per core (phase 2)

LAST_TIMINGS = {}

# --------------------------------------------------------------------------
# phase 1 program
# --------------------------------------------------------------------------


def build_phase1():
    nc = bacc.Bacc("TRN2", target_bir_lowering=False, debug=False,
                   enable_asserts=False, num_devices=NCORES)
    x_d = nc.dram_tensor("x", [RPC, E], dt.float32, kind="ExternalInput").ap()
    w1_d = nc.dram_tensor("w1", [E, DF], dt.bfloat16, kind="ExternalInput").ap()
    w2_d = nc.dram_tensor("w2", [DF, E], dt.bfloat16, kind="ExternalInput").ap()
    b1_d = nc.dram_tensor("b1", [128, 32], dt.float32, kind="ExternalInput").ap()
    b2_d = nc.dram_tensor("b2", [128, E], dt.float32, kind="ExternalInput").ap()
    tng_d = nc.dram_tensor("tng", [128, E], dt.bfloat16, kind="ExternalInput").ap()
    tnb_d = nc.dram_tensor("tnb", [128, E], dt.bfloat16, kind="ExternalInput").ap()
    id_d = nc.dram_tensor("ident", [128, 128], dt.bfloat16, kind="ExternalInput").ap()
    y_d = nc.dram_tensor("y", [RPC, E], dt.float32, kind="ExternalOutput").ap()
    h2_d = nc.dram_tensor("h2", [RPC, E], dt.bfloat16, kind="ExternalOutput").ap()

    NT = 4          # token tiles per block (block = 512 tokens)
    NBLK = RPC // (128 * NT)   # 2 blocks

    from contextlib import ExitStack
    with tile.TileContext(nc) as tc, ExitStack() as es:
        pool = lambda **kw: es.enter_context(tc.tile_pool(**kw))
        constp = pool(name="const", bufs=1)
        w1p = pool(name="w1p", bufs=8)
        xp = pool(name="xp", bufs=2)
        xrp = pool(name="xrp", bufs=4)
        xcp = pool(name="xcp", bufs=2)
        sqp = pool(name="sqp", bufs=2)
        statp = pool(name="stat", bufs=24)
        hbfp = pool(name="hbf", bufs=2)
        htp = pool(name="htp", bufs=17)
        atp = pool(name="atp", bufs=33)
        w2p = pool(name="w2p", bufs=3)
        yp = pool(name="yp", bufs=5)
        h2p = pool(name="h2p", bufs=2)
        mps = pool(name="mps", bufs=8, space="PSUM")
        if True:
            # small consts first so PE-side deps clear fast
            id_sb = constp.tile([128, 128], dt.bfloat16, tag="ident")
            nc.sync.dma_start(out=id_sb[:, :], in_=id_d[:, :])
            eps_sb = constp.tile([128, 1], dt.float32, tag="eps")
            nc.gpsimd.memset(eps_sb[:, :], EPS)

            def layernorm_to(src, dst_bf, scale_rows=None, bias_rows=None):
                """dst_bf (bf16) = LN(src) [* scale_rows + bias_rows]."""
                ssum = statp.tile([128, 1], dt.float32, tag="ssum")
                nc.vector.reduce_sum(ssum[:, :], src[:, :], axis=AX.X)
                negmean = statp.tile([128, 1], dt.float32, tag="negmean")
                nc.scalar.mul(negmean[:, :], ssum[:, :], -1.0 / E)
                xc = xcp.tile([128, E], dt.float32, tag="xc")
                nc.vector.tensor_scalar_add(xc[:, :], src[:, :], negmean[:, :])
                sq = sqp.tile([128, E], dt.bfloat16, tag="sq")
                ssq = statp.tile([128, 1], dt.float32, tag="ssq")
                nc.scalar.activation(sq[:, :], xc[:, :], AF.Square,
                                     accum_out=ssq[:, :])
                std = statp.tile([128, 1], dt.float32, tag="std")
                nc.scalar.activation(std[:, :], ssq[:, :], AF.Sqrt,
                                     scale=1.0 / E, bias=eps_sb[:, :])
                rstd = statp.tile([128, 1], dt.float32, tag="rstd")
                nc.vector.reciprocal(rstd[:, :], std[:, :])
                nc.scalar.activation(dst_bf[:, :], xc[:, :], AF.Copy,
                                     scale=rstd[:, :])
                if scale_rows is not None:
                    nc.vector.tensor_mul(dst_bf[:, :], dst_bf[:, :],
                                         scale_rows[:, :])
                if bias_rows is not None:
                    nc.vector.tensor_add(dst_bf[:, :], dst_bf[:, :],
                                         bias_rows[:, :])

            # ---- LN1 + transpose for ALL tokens up front ----
            # hT[blk][e] : [e 128, tok 512] bf16
            hT = [[None] * 8 for _ in range(NBLK)]
            for blk in range(NBLK):
                row0 = blk * 128 * NT
                for tt in range(NT):
                    xt = xp.tile([128, E], dt.float32, tag="xt")
                    nc.sync.dma_start(
                        out=xt[:, :],
                        in_=x_d[row0 + tt * 128: row0 + (tt + 1) * 128, :])
                    hb = hbfp.tile([128, E], dt.bfloat16, tag="hb")
                    layernorm_to(xt, hb)
                    for e in range(8):
                        pt = mps.tile([128, 128], dt.bfloat16, tag="mp",
                                      name=f"tp{blk}_{tt}_{e}")
                        nc.tensor.transpose(
                            pt[:, :], hb[:, e * 128:(e + 1) * 128], id_sb[:, :])
                        if hT[blk][e] is None:
                            hT[blk][e] = htp.tile([128, 512], dt.bfloat16,
                                                  tag="ht", name=f"ht{blk}_{e}")
                        nc.vector.tensor_copy(
                            hT[blk][e][:, tt * 128:(tt + 1) * 128], pt[:, :])

            # ---- weights (after x in DMA program order) ----
            w1_sb = []
            for i in range(8):
                t = w1p.tile([128, DF], dt.bfloat16, tag="w1sb")
                nc.sync.dma_start(out=t[:, :], in_=w1_d[i * 128:(i + 1) * 128, :])
                w1_sb.append(t)
            b1_sb = constp.tile([128, 32], dt.float32, tag="b1")
            nc.sync.dma_start(out=b1_sb[:, :], in_=b1_d[:, :])
            b2_sb = constp.tile([128, E], dt.float32, tag="b2")
            nc.sync.dma_start(out=b2_sb[:, :], in_=b2_d[:, :])
            tng_sb = constp.tile([128, E], dt.bfloat16, tag="tng")
            nc.sync.dma_start(out=tng_sb[:, :], in_=tng_d[:, :])
            tnb_sb = constp.tile([128, E], dt.bfloat16, tag="tnb")
            nc.sync.dma_start(out=tnb_sb[:, :], in_=tnb_d[:, :])

            for blk in range(NBLK):
                row0 = blk * 128 * NT
                # ---- mm1 + silu -> aT[df][df 128, tok 512] (bf16) ----
                aT = []
                for df in range(32):
                    ps = mps.tile([128, 512], dt.float32, tag="mp",
                                  name=f"m1_{blk}_{df}")
                    for e in range(8):
                        nc.tensor.matmul(
                            ps[:, :],
                            w1_sb[e][:, df * 128:(df + 1) * 128],
                            hT[blk][e][:, :],
                            start=(e == 0), stop=(e == 7))
                    at = atp.tile([128, 512], dt.bfloat16, tag="at")
                    nc.scalar.activation(at[:, :], ps[:, :], AF.Silu,
                                         bias=b1_sb[:, df:df + 1])
                    aT.append(at)
                # ---- prefetch residual x slices for this block ----
                xr_t = []
                for tt in range(NT):
                    xr = xrp.tile([128, E], dt.float32, tag="xr",
                                  name=f"xr{blk}_{tt}")
                    nc.sync.dma_start(
                        out=xr[:, :],
                        in_=x_d[row0 + tt * 128: row0 + (tt + 1) * 128, :])
                    xr_t.append(xr)
                # ---- mm2: df-outer, stream full W2 rows; 8 psum banks ----
                pss = [mps.tile([128, 512], dt.float32, tag="mp",
                                name=f"m2_{blk}_{i}") for i in range(8)]
                for df in range(32):
                    w2t = w2p.tile([128, E], dt.bfloat16, tag="w2t")
                    nc.sync.dma_start(
                        out=w2t[:, :], in_=w2_d[df * 128:(df + 1) * 128, :])
                    for tt in range(NT):
                        for eb in range(2):
                            nc.tensor.matmul(
                                pss[tt * 2 + eb][:, :],
                                aT[df][:, tt * 128:(tt + 1) * 128],
                                w2t[:, eb * 512:(eb + 1) * 512],
                                start=(df == 0), stop=(df == 31))
                # ---- drain: y = psum + x + b2; write y; LN2 -> h2 ----
                for tt in range(NT):
                    y_t = yp.tile([128, E], dt.float32, tag="yt",
                                  name=f"yt{blk}_{tt}")
                    for eb in range(2):
                        ysl = y_t[:, eb * 512:(eb + 1) * 512]
                        nc.vector.tensor_add(
                            ysl, pss[tt * 2 + eb][:, :],
                            xr_t[tt][:, eb * 512:(eb + 1) * 512])
                        nc.gpsimd.tensor_add(
                            ysl, ysl, b2_sb[:, eb * 512:(eb + 1) * 512])
                    nc.sync.dma_start(
                        out=y_d[row0 + tt * 128: row0 + (tt + 1) * 128, :],
                        in_=y_t[:, :])
                    h2t = h2p.tile([128, E], dt.bfloat16, tag="h2t")
                    layernorm_to(y_t, h2t, scale_rows=tng_sb,
                                 bias_rows=tnb_sb)
                    nc.sync.dma_start(
                        out=h2_d[row0 + tt * 128: row0 + (tt + 1) * 128, :],
                        in_=h2t[:, :])
    nc.compile()
    nc.m = get_hw_module(nc.m)
    return nc


# --------------------------------------------------------------------------
# phase 2 program
# --------------------------------------------------------------------------


def build_phase2():
    nc = bacc.Bacc("TRN2", target_bir_lowering=False, debug=False,
                   enable_asserts=False, num_devices=NCORES)
    # packed layouts: hs_d[p, t*BE + be] = h2T[t*128+p, be]
    #                 r_d[p, d*512 + j] = R[d][p, j]
    hs_d = nc.dram_tensor("h2t", [128, 32 * BE], dt.bfloat16, kind="ExternalInput").ap()
    r_d = nc.dram_tensor("rt", [128, 32 * 512], dt.bfloat16, kind="ExternalInput").ap()
    yt_d = nc.dram_tensor("yt", [BE, S], dt.float32, kind="ExternalInput").ap()
    tb_d = nc.dram_tensor("tb", [1, S], dt.bfloat16, kind="ExternalInput").ap()
    ones_d = nc.dram_tensor("ones", [1, 128], dt.bfloat16, kind="ExternalInput").ap()
    out_d = nc.dram_tensor("out", [BE, S], dt.float32, kind="ExternalOutput").ap()

    from contextlib import ExitStack
    with tile.TileContext(nc) as tc, ExitStack() as es:
        pool = lambda **kw: es.enter_context(tc.tile_pool(**kw))
        hsp = pool(name="hs", bufs=4)
        rtp = pool(name="rt", bufs=8)
        constp = pool(name="const", bufs=1)
        yinp = pool(name="yin", bufs=6)
        outp = pool(name="outp", bufs=6)
        psp = pool(name="ps", bufs=8, space="PSUM")
        if True:
            tb_sb = constp.tile([1, S], dt.bfloat16, tag="tb")
            nc.sync.dma_start(out=tb_sb[:, :], in_=tb_d[:, :])
            ones_sb = constp.tile([1, 128], dt.bfloat16, tag="ones")
            nc.sync.dma_start(out=ones_sb[:, :], in_=ones_d[:, :])
            # chunked loads in consumption order: rt chunk g covers d=4g..4g+3,
            # hs chunk c covers t=8c..8c+7
            hs_t = [None] * 4   # [128, 2048] each
            rt_t = [None] * 8   # [128, 2048] each
            order = [("rt", 0), ("hs", 0), ("rt", 1), ("rt", 2), ("hs", 1),
                     ("rt", 3), ("rt", 4), ("hs", 2), ("rt", 5), ("rt", 6),
                     ("hs", 3), ("rt", 7)]
            for kind, c in order:
                if kind == "rt":
                    rt_t[c] = rtp.tile([128, 2048], dt.bfloat16, tag="rt",
                                       name=f"rt{c}")
                    nc.sync.dma_start(out=rt_t[c][:, :],
                                      in_=r_d[:, c * 2048:(c + 1) * 2048])
                else:
                    hs_t[c] = hsp.tile([128, 2048], dt.bfloat16, tag="hs",
                                       name=f"hs{c}")
                    nc.sync.dma_start(out=hs_t[c][:, :],
                                      in_=hs_d[:, c * 2048:(c + 1) * 2048])

            def hs_ap(t, be):
                return hs_t[t // 8][:, (t % 8) * BE + be * 128:
                                    (t % 8) * BE + be * 128 + 128]

            def rt_ap(d):
                return rt_t[d // 4][:, (d % 4) * 512:(d % 4) * 512 + 512]

            for sb in range(8):
                for be in range(2):
                    ps = psp.tile([128, 512], dt.float32, tag="ps",
                                  name=f"ps{sb}_{be}")
                    for t in range(4 * sb + 4):
                        d = 4 * sb - t + 3
                        nc.tensor.matmul(
                            ps[:, :], hs_ap(t, be), rt_ap(d),
                            start=(t == 0), stop=False)
                    nc.tensor.matmul(
                        ps[:, :], ones_sb[:, :],
                        tb_sb[:, sb * 512:(sb + 1) * 512],
                        start=False, stop=True)
                    yin = yinp.tile([128, 512], dt.float32, tag="yin")
                    nc.sync.dma_start(
                        out=yin[:, :],
                        in_=yt_d[be * 128:(be + 1) * 128,
                                 sb * 512:(sb + 1) * 512])
                    ot = outp.tile([128, 512], dt.float32, tag="ot")
                    nc.vector.tensor_add(ot[:, :], ps[:, :], yin[:, :])
                    nc.sync.dma_start(
                        out=out_d[be * 128:(be + 1) * 128,
                                  sb * 512:(sb + 1) * 512],
                        in_=ot[:, :])
    nc.compile()
    nc.m = get_hw_module(nc.m)
    return nc


def _install_ntff_hook():
    """The agent image's antenv lacks axon_hooks; synthesize it so
    run_bass_kernel_spmd(trace=True) can capture NTFF profiles."""
    import types
    import antenv

    if "antenv.axon_hooks" in sys.modules:
        return
    mod = types.ModuleType("antenv.axon_hooks")
    state = {"h": None}
    mod.set_axon_ntff_profile_hook = lambda h: state.__setitem__("h", h)
    mod.get_axon_ntff_profile_hook = lambda: state["h"]
    sys.modules["antenv.axon_hooks"] = mod
    antenv.axon_hooks = mod
    from trn_agent_boot.trn_boot import _ntff_profile_via_ctypes

    mod.set_axon_ntff_profile_hook(
        _ntff_profile_via_ctypes("/opt/axon/libaxon_pjrt.so"))
    bass_utils.upload_artifacts = lambda tmpdir: tmpdir


_P1 = None
_P2 = None


def _programs():
    global _P1, _P2
    if _P1 is None:
        _P1 = build_phase1()
    if _P2 is None:
        _P2 = build_phase2()
    return _P1, _P2


def _run(nc, in_maps, trace):
    if trace:
        try:
            _install_ntff_hook()
        except Exception as e:
            print(f"ntff hook install failed: {e}", file=sys.stderr)
            trace = False
    res = bass_utils.run_bass_kernel_spmd(
        nc, in_maps, core_ids=list(range(NCORES)), trace=trace)
    return res


def kernel(x, cn_g, cn_b, W1, b1, W2, b2, tn_g, tn_b, tw, tb):
    trace = os.environ.get("MIXER_TRACE", "0") == "1"
    x = np.asarray(x, np.float32)
    p1, p2 = _programs()

    # ---- host prep (inputs only) ----
    W1 = np.asarray(W1, np.float32)
    W2 = np.asarray(W2, np.float32)
    cn_g = np.asarray(cn_g, np.float32)
    cn_b = np.asarray(cn_b, np.float32)
    w1g = (cn_g[:, None] * W1).astype(BF16)
    b1f = (np.asarray(b1, np.float32) + cn_b @ W1).astype(np.float32)
    b1_t = np.ascontiguousarray(b1f.reshape(32, 128).T)          # [128, 32]
    w2bf = W2.astype(BF16)
    b2b = np.ascontiguousarray(
        np.broadcast_to(np.asarray(b2, np.float32), (128, E)))
    tngb = np.ascontiguousarray(
        np.broadcast_to(np.asarray(tn_g, np.float32).astype(BF16), (128, E)))
    tnbb = np.ascontiguousarray(
        np.broadcast_to(np.asarray(tn_b, np.float32).astype(BF16), (128, E)))
    ident = np.eye(128, dtype=BF16)

    xf = x.reshape(B * S, E)
    in_maps1 = []
    for c in range(NCORES):
        in_maps1.append({
            "x": np.ascontiguousarray(xf[c * RPC:(c + 1) * RPC]),
            "w1": w1g, "w2": w2bf, "b1": b1_t, "b2": b2b,
            "tng": tngb, "tnb": tnbb, "ident": ident,
        })
    r1 = _run(p1, in_maps1, trace)
    if trace:
        LAST_TIMINGS["phase1_ns"] = r1.exec_time_ns
    y = np.concatenate([np.asarray(r1.results[c]["y"], np.float32)
                        for c in range(NCORES)], axis=0)
    h2 = np.concatenate([np.asarray(r1.results[c]["h2"]).view(BF16)
                         if r1.results[c]["h2"].dtype != BF16
                         else r1.results[c]["h2"]
                         for c in range(NCORES)], axis=0)

    # ---- phase 2 host glue ----
    tw = np.asarray(tw, np.float32)
    pad = np.zeros(512 + S + 512, np.float32)
    pad[512:512 + S] = tw
    # R[d][i, j] = tw_ext[(d-3)*128 + j - i]
    win = np.lib.stride_tricks.sliding_window_view(pad, 512)   # win[k] = pad[k:k+512]
    rtiles = np.empty((32, 128, 512), np.float32)
    ii = np.arange(128)
    for d in range(32):
        rtiles[d] = win[512 + (d - 3) * 128 - ii]
    rtiles_bf = np.ascontiguousarray(
        rtiles.astype(BF16).transpose(1, 0, 2).reshape(128, 32 * 512))
    tb_row = np.asarray(tb, np.float32).astype(BF16).reshape(1, S)
    ones_row = np.ones((1, 128), BF16)

    h2v = h2.reshape(B, S, E)
    yv = y.reshape(B, S, E)
    in_maps2 = []
    for c in range(NCORES):
        e0 = c * EPC
        h2sl = np.ascontiguousarray(
            h2v[:, :, e0:e0 + EPC].transpose(1, 0, 2).reshape(32, 128, BE)
            .transpose(1, 0, 2).reshape(128, 32 * BE))
        ysl = np.ascontiguousarray(
            yv[:, :, e0:e0 + EPC].transpose(0, 2, 1).reshape(BE, S))
        in_maps2.append({"h2t": h2sl, "rt": rtiles_bf, "yt": ysl,
                         "tb": tb_row, "ones": ones_row})
    r2 = _run(p2, in_maps2, trace)
    if trace:
        LAST_TIMINGS["phase2_ns"] = r2.exec_time_ns

    out = np.empty((B, S, E), np.float32)
    for c in range(NCORES):
        e0 = c * EPC
        o = np.asarray(r2.results[c]["out"], np.float32).reshape(B, EPC, S)
        out[:, :, e0:e0 + EPC] = o.transpose(0, 2, 1)
    return out


# revision 16
# speedup vs baseline: 1.2142x; 1.0903x over previous
"""MixerBlock TRN2 kernel: B=2, S=4096, E=1024, DF=4096 on 8 NeuronCores.

Strategy (two SPMD launches):
  Phase 1 (shard B*S=8192 rows -> 1024 rows/core):
    h   = LN(x)            (cn affine folded into W1/b1 host-side)
    a   = silu(h @ W1g + b1')        -> kept transposed aT[df, tok]
    y   = x + aT.T @ W2 + b2
    h2  = LN(y)*tn_g + tn_b          (bf16)
    outputs y (f32), h2 (bf16)
  Phase 2 (shard E=1024 -> 128 channels/core; rows (b,e) = 256/core):
    out[be, s] = sum_t h2T[t, be] * M[t, s] + tb[s] + y[be, s]
    The Toeplitz matrix M[t,s] = tw[s-t] (s>=t) is diagonal-constant, so a
    [128t x 512s] tile depends only on (512*sb - 128*t): 32 distinct tiles,
    prebuilt host-side from tw (4 MB bf16), used as the moving operand.
"""

import os
import sys

sys.path.insert(0, "/opt/trn_rl_repo")
sys.path.insert(0, "/opt/trn_rl_repo/concourse")

import numpy as np
import ml_dtypes

import concourse.bass as bass
import concourse.bacc as bacc
import concourse.mybir as mybir
from concourse import tile
from concourse import bass_utils
from concourse.bass_interp import get_hw_module

dt = mybir.dt
AF = mybir.ActivationFunctionType
AX = mybir.AxisListType
BF16 = ml_dtypes.bfloat16

B, S, E = 2, 4096, 1024
DF = 4 * E
EPS = 1e-5
NCORES = 8
RPC = (B * S) // NCORES      # 1024 rows per core (phase 1)
EPC = E // NCORES            # 128 channels per core (phase 2)
BE = B * EPC                 # 256 (b,e) rows per core (phase 2)

LAST_TIMINGS = {}

# --------------------------------------------------------------------------
# phase 1 program
# --------------------------------------------------------------------------


def build_phase1():
    nc = bacc.Bacc("TRN2", target_bir_lowering=False, debug=False,
                   enable_asserts=False, num_devices=NCORES)
    x_d = nc.dram_tensor("x", [RPC, E], dt.float32, kind="ExternalInput").ap()
    w1_d = nc.dram_tensor("w1", [E, DF], dt.bfloat16, kind="ExternalInput").ap()
    w2_d = nc.dram_tensor("w2", [DF, E], dt.bfloat16, kind="ExternalInput").ap()
    b1_d = nc.dram_tensor("b1", [128, 32], dt.float32, kind="ExternalInput").ap()
    b2_d = nc.dram_tensor("b2", [128, E], dt.float32, kind="ExternalInput").ap()
    id_d = nc.dram_tensor("ident", [128, 128], dt.bfloat16, kind="ExternalInput").ap()
    y_d = nc.dram_tensor("y", [RPC, E], dt.float32, kind="ExternalOutput").ap()
    h2_d = nc.dram_tensor("h2", [RPC, E], dt.bfloat16, kind="ExternalOutput").ap()

    NT = 4          # token tiles per block (block = 512 tokens)
    NBLK = RPC // (128 * NT)   # 2 blocks

    from contextlib import ExitStack
    with tile.TileContext(nc) as tc, ExitStack() as es:
        pool = lambda **kw: es.enter_context(tc.tile_pool(**kw))
        constp = pool(name="const", bufs=1)
        w1p = pool(name="w1p", bufs=8)
        xp = pool(name="xp", bufs=4)
        xrp = pool(name="xrp", bufs=4)
        statp = pool(name="stat", bufs=24)
        hbfp = pool(name="hbf", bufs=2)
        htp = pool(name="htp", bufs=17)
        atp = pool(name="atp", bufs=33)
        w2p = pool(name="w2p", bufs=4)
        yp = pool(name="yp", bufs=5)
        h2p = pool(name="h2p", bufs=2)
        mps = pool(name="mps", bufs=8, space="PSUM")
        if True:
            # small consts first so PE-side deps clear fast
            id_sb = constp.tile([128, 128], dt.bfloat16, tag="ident")
            nc.sync.dma_start(out=id_sb[:, :], in_=id_d[:, :])
            eps_sb = constp.tile([128, 1], dt.float32, tag="eps")
            nc.gpsimd.memset(eps_sb[:, :], EPS)

            def layernorm_to(src, dst_bf):
                """dst_bf (bf16) = (src - mean) / sqrt(var + eps), rowwise."""
                stats = statp.tile([128, 2, 6], dt.float32, tag="bst")
                for i in range(2):
                    nc.vector.bn_stats(stats[:, i, :],
                                       src[:, i * 512:(i + 1) * 512])
                mv = statp.tile([128, 2], dt.float32, tag="mv")
                nc.vector.bn_aggr(mv[:, :], stats[:, :, :])
                nc.scalar.activation(mv[:, 1:2], mv[:, 1:2], AF.Sqrt,
                                     scale=1.0, bias=eps_sb[:, :])
                nc.vector.reciprocal(mv[:, 1:2], mv[:, 1:2])
                nc.vector.tensor_scalar(dst_bf[:, :], src[:, :],
                                        mv[:, 0:1], mv[:, 1:2],
                                        op0=mybir.AluOpType.subtract,
                                        op1=mybir.AluOpType.mult)

            # ---- LN1 + transpose for ALL tokens up front ----
            # hT[blk][e] : [e 128, tok 512] bf16
            hT = [[None] * 8 for _ in range(NBLK)]
            for blk in range(NBLK):
                row0 = blk * 128 * NT
                for tt in range(NT):
                    xt = xp.tile([128, E], dt.float32, tag="xt")
                    nc.sync.dma_start(
                        out=xt[:, :],
                        in_=x_d[row0 + tt * 128: row0 + (tt + 1) * 128, :])
                    hb = hbfp.tile([128, E], dt.bfloat16, tag="hb")
                    layernorm_to(xt, hb)
                    for e in range(8):
                        pt = mps.tile([128, 128], dt.bfloat16, tag="mp",
                                      name=f"tp{blk}_{tt}_{e}")
                        nc.tensor.transpose(
                            pt[:, :], hb[:, e * 128:(e + 1) * 128], id_sb[:, :])
                        if hT[blk][e] is None:
                            hT[blk][e] = htp.tile([128, 512], dt.bfloat16,
                                                  tag="ht", name=f"ht{blk}_{e}")
                        nc.vector.tensor_copy(
                            hT[blk][e][:, tt * 128:(tt + 1) * 128], pt[:, :])

            # ---- weights (after x in DMA program order) ----
            w1_sb = []
            for i in range(8):
                t = w1p.tile([128, DF], dt.bfloat16, tag="w1sb")
                nc.sync.dma_start(out=t[:, :], in_=w1_d[i * 128:(i + 1) * 128, :])
                w1_sb.append(t)
            b1_sb = constp.tile([128, 32], dt.float32, tag="b1")
            nc.sync.dma_start(out=b1_sb[:, :], in_=b1_d[:, :])
            b2_sb = constp.tile([128, E], dt.float32, tag="b2")
            nc.sync.dma_start(out=b2_sb[:, :], in_=b2_d[:, :])

            for blk in range(NBLK):
                row0 = blk * 128 * NT
                # ---- mm1 + silu -> aT[df][df 128, tok 512] (bf16) ----
                aT = []
                for df in range(32):
                    ps = mps.tile([128, 512], dt.float32, tag="mp",
                                  name=f"m1_{blk}_{df}")
                    for e in range(8):
                        nc.tensor.matmul(
                            ps[:, :],
                            w1_sb[e][:, df * 128:(df + 1) * 128],
                            hT[blk][e][:, :],
                            start=(e == 0), stop=(e == 7))
                    at = atp.tile([128, 512], dt.bfloat16, tag="at")
                    nc.scalar.activation(at[:, :], ps[:, :], AF.Silu,
                                         bias=b1_sb[:, df:df + 1])
                    aT.append(at)
                # ---- prefetch residual x slices for this block ----
                xr_t = []
                for tt in range(NT):
                    xr = xrp.tile([128, E], dt.float32, tag="xr",
                                  name=f"xr{blk}_{tt}")
                    nc.sync.dma_start(
                        out=xr[:, :],
                        in_=x_d[row0 + tt * 128: row0 + (tt + 1) * 128, :])
                    xr_t.append(xr)
                # ---- mm2: df-outer, stream full W2 rows; 8 psum banks ----
                pss = [mps.tile([128, 512], dt.float32, tag="mp",
                                name=f"m2_{blk}_{i}") for i in range(8)]
                for df in range(32):
                    w2t = w2p.tile([128, E], dt.bfloat16, tag="w2t")
                    nc.sync.dma_start(
                        out=w2t[:, :], in_=w2_d[df * 128:(df + 1) * 128, :])
                    for tt in range(NT):
                        for eb in range(2):
                            nc.tensor.matmul(
                                pss[tt * 2 + eb][:, :],
                                aT[df][:, tt * 128:(tt + 1) * 128],
                                w2t[:, eb * 512:(eb + 1) * 512],
                                start=(df == 0), stop=(df == 31))
                # ---- drain: y = psum + x + b2; write y; LN2 -> h2 ----
                for tt in range(NT):
                    y_t = yp.tile([128, E], dt.float32, tag="yt",
                                  name=f"yt{blk}_{tt}")
                    for eb in range(2):
                        ysl = y_t[:, eb * 512:(eb + 1) * 512]
                        nc.vector.tensor_add(
                            ysl, pss[tt * 2 + eb][:, :],
                            xr_t[tt][:, eb * 512:(eb + 1) * 512])
                        nc.gpsimd.tensor_add(
                            ysl, ysl, b2_sb[:, eb * 512:(eb + 1) * 512])
                    nc.gpsimd.dma_start(
                        out=y_d[row0 + tt * 128: row0 + (tt + 1) * 128, :],
                        in_=y_t[:, :])
                    h2t = h2p.tile([128, E], dt.bfloat16, tag="h2t")
                    layernorm_to(y_t, h2t)
                    nc.gpsimd.dma_start(
                        out=h2_d[row0 + tt * 128: row0 + (tt + 1) * 128, :],
                        in_=h2t[:, :])
    nc.compile()
    nc.m = get_hw_module(nc.m)
    return nc


# --------------------------------------------------------------------------
# phase 2 program
# --------------------------------------------------------------------------


def build_phase2():
    nc = bacc.Bacc("TRN2", target_bir_lowering=False, debug=False,
                   enable_asserts=False, num_devices=NCORES)
    # packed layouts: hs_d[p, t*BE + be] = h2T[t*128+p, be]
    #                 r_d[p, d*512 + j] = R[d][p, j]
    hs_d = nc.dram_tensor("h2t", [128, 32 * BE], dt.bfloat16, kind="ExternalInput").ap()
    r_d = nc.dram_tensor("rt", [128, 32 * 512], dt.bfloat16, kind="ExternalInput").ap()
    yt_d = nc.dram_tensor("yt", [BE, S], dt.float32, kind="ExternalInput").ap()
    tb_d = nc.dram_tensor("tb", [1, S], dt.bfloat16, kind="ExternalInput").ap()
    ones_d = nc.dram_tensor("ones", [1, 128], dt.bfloat16, kind="ExternalInput").ap()
    bg_d = nc.dram_tensor("bg", [1, 128], dt.bfloat16, kind="ExternalInput").ap()
    csum_d = nc.dram_tensor("csum", [1, S], dt.bfloat16, kind="ExternalInput").ap()
    gcol_d = nc.dram_tensor("gcol", [128, 1], dt.float32, kind="ExternalInput").ap()
    out_d = nc.dram_tensor("out", [BE, S], dt.float32, kind="ExternalOutput").ap()

    from contextlib import ExitStack
    with tile.TileContext(nc) as tc, ExitStack() as es:
        pool = lambda **kw: es.enter_context(tc.tile_pool(**kw))
        hsp = pool(name="hs", bufs=4)
        rtp = pool(name="rt", bufs=8)
        constp = pool(name="const", bufs=1)
        yinp = pool(name="yin", bufs=6)
        outp = pool(name="outp", bufs=6)
        psp = pool(name="ps", bufs=8, space="PSUM")
        if True:
            tb_sb = constp.tile([1, S], dt.bfloat16, tag="tb")
            nc.sync.dma_start(out=tb_sb[:, :], in_=tb_d[:, :])
            ones_sb = constp.tile([1, 128], dt.bfloat16, tag="ones")
            nc.sync.dma_start(out=ones_sb[:, :], in_=ones_d[:, :])
            bg_sb = constp.tile([1, 128], dt.bfloat16, tag="bg")
            nc.sync.dma_start(out=bg_sb[:, :], in_=bg_d[:, :])
            csum_sb = constp.tile([1, S], dt.bfloat16, tag="csum")
            nc.sync.dma_start(out=csum_sb[:, :], in_=csum_d[:, :])
            gcol_sb = constp.tile([128, 1], dt.float32, tag="gcol")
            nc.sync.dma_start(out=gcol_sb[:, :], in_=gcol_d[:, :])
            # chunked loads in consumption order: rt chunk g covers d=4g..4g+3,
            # hs chunk c covers t=8c..8c+7
            hs_t = [None] * 4   # [128, 2048] each
            rt_t = [None] * 8   # [128, 2048] each
            order = [("rt", 0), ("hs", 0), ("rt", 1), ("rt", 2), ("hs", 1),
                     ("rt", 3), ("rt", 4), ("hs", 2), ("rt", 5), ("rt", 6),
                     ("hs", 3), ("rt", 7)]
            for kind, c in order:
                if kind == "rt":
                    rt_t[c] = rtp.tile([128, 2048], dt.bfloat16, tag="rt",
                                       name=f"rt{c}")
                    nc.sync.dma_start(out=rt_t[c][:, :],
                                      in_=r_d[:, c * 2048:(c + 1) * 2048])
                else:
                    hs_t[c] = hsp.tile([128, 2048], dt.bfloat16, tag="hs",
                                       name=f"hs{c}")
                    nc.sync.dma_start(out=hs_t[c][:, :],
                                      in_=hs_d[:, c * 2048:(c + 1) * 2048])

            def hs_ap(t, be):
                return hs_t[t // 8][:, (t % 8) * BE + be * 128:
                                    (t % 8) * BE + be * 128 + 128]

            def rt_ap(d):
                return rt_t[d // 4][:, (d % 4) * 512:(d % 4) * 512 + 512]

            for sb in range(8):
                for be in range(2):
                    ps = psp.tile([128, 512], dt.float32, tag="ps",
                                  name=f"ps{sb}_{be}")
                    for t in range(4 * sb + 4):
                        d = 4 * sb - t + 3
                        nc.tensor.matmul(
                            ps[:, :], hs_ap(t, be), rt_ap(d),
                            start=(t == 0), stop=False)
                    nc.tensor.matmul(
                        ps[:, :], ones_sb[:, :],
                        tb_sb[:, sb * 512:(sb + 1) * 512],
                        start=False, stop=False)
                    nc.tensor.matmul(
                        ps[:, :], bg_sb[:, :],
                        csum_sb[:, sb * 512:(sb + 1) * 512],
                        start=False, stop=True)
                    yin = yinp.tile([128, 512], dt.float32, tag="yin")
                    nc.sync.dma_start(
                        out=yin[:, :],
                        in_=yt_d[be * 128:(be + 1) * 128,
                                 sb * 512:(sb + 1) * 512])
                    ot = outp.tile([128, 512], dt.float32, tag="ot")
                    nc.vector.scalar_tensor_tensor(
                        ot[:, :], ps[:, :], gcol_sb[:, 0:1], yin[:, :],
                        op0=mybir.AluOpType.mult, op1=mybir.AluOpType.add)
                    nc.sync.dma_start(
                        out=out_d[be * 128:(be + 1) * 128,
                                  sb * 512:(sb + 1) * 512],
                        in_=ot[:, :])
    nc.compile()
    nc.m = get_hw_module(nc.m)
    return nc


def _install_ntff_hook():
    """The agent image's antenv lacks axon_hooks; synthesize it so
    run_bass_kernel_spmd(trace=True) can capture NTFF profiles."""
    import types
    import antenv

    if "antenv.axon_hooks" in sys.modules:
        return
    mod = types.ModuleType("antenv.axon_hooks")
    state = {"h": None}
    mod.set_axon_ntff_profile_hook = lambda h: state.__setitem__("h", h)
    mod.get_axon_ntff_profile_hook = lambda: state["h"]
    sys.modules["antenv.axon_hooks"] = mod
    antenv.axon_hooks = mod
    from trn_agent_boot.trn_boot import _ntff_profile_via_ctypes

    mod.set_axon_ntff_profile_hook(
        _ntff_profile_via_ctypes("/opt/axon/libaxon_pjrt.so"))
    bass_utils.upload_artifacts = lambda tmpdir: tmpdir


_P1 = None
_P2 = None


def _programs():
    global _P1, _P2
    if _P1 is None:
        _P1 = build_phase1()
    if _P2 is None:
        _P2 = build_phase2()
    return _P1, _P2


def _run(nc, in_maps, trace):
    if trace:
        try:
            _install_ntff_hook()
        except Exception as e:
            print(f"ntff hook install failed: {e}", file=sys.stderr)
            trace = False
    res = bass_utils.run_bass_kernel_spmd(
        nc, in_maps, core_ids=list(range(NCORES)), trace=trace)
    return res


def kernel(x, cn_g, cn_b, W1, b1, W2, b2, tn_g, tn_b, tw, tb):
    trace = os.environ.get("MIXER_TRACE", "0") == "1"
    x = np.asarray(x, np.float32)
    p1, p2 = _programs()

    # ---- host prep (inputs only) ----
    W1 = np.asarray(W1, np.float32)
    W2 = np.asarray(W2, np.float32)
    cn_g = np.asarray(cn_g, np.float32)
    cn_b = np.asarray(cn_b, np.float32)
    w1g = (cn_g[:, None] * W1).astype(BF16)
    b1f = (np.asarray(b1, np.float32) + cn_b @ W1).astype(np.float32)
    b1_t = np.ascontiguousarray(b1f.reshape(32, 128).T)          # [128, 32]
    w2bf = W2.astype(BF16)
    b2b = np.ascontiguousarray(
        np.broadcast_to(np.asarray(b2, np.float32), (128, E)))
    ident = np.eye(128, dtype=BF16)
    tn_g = np.asarray(tn_g, np.float32)
    tn_b = np.asarray(tn_b, np.float32)

    xf = x.reshape(B * S, E)
    in_maps1 = []
    for c in range(NCORES):
        in_maps1.append({
            "x": np.ascontiguousarray(xf[c * RPC:(c + 1) * RPC]),
            "w1": w1g, "w2": w2bf, "b1": b1_t, "b2": b2b,
            "ident": ident,
        })
    r1 = _run(p1, in_maps1, trace)
    if trace:
        LAST_TIMINGS["phase1_ns"] = r1.exec_time_ns
    y = np.concatenate([np.asarray(r1.results[c]["y"], np.float32)
                        for c in range(NCORES)], axis=0)
    h2 = np.concatenate([np.asarray(r1.results[c]["h2"]).view(BF16)
                         if r1.results[c]["h2"].dtype != BF16
                         else r1.results[c]["h2"]
                         for c in range(NCORES)], axis=0)

    # ---- phase 2 host glue ----
    tw = np.asarray(tw, np.float32)
    pad = np.zeros(512 + S + 512, np.float32)
    pad[512:512 + S] = tw
    # R[d][i, j] = tw_ext[(d-3)*128 + j - i]
    win = np.lib.stride_tricks.sliding_window_view(pad, 512)   # win[k] = pad[k:k+512]
    rtiles = np.empty((32, 128, 512), np.float32)
    ii = np.arange(128)
    for d in range(32):
        rtiles[d] = win[512 + (d - 3) * 128 - ii]
    rtiles_bf = np.ascontiguousarray(
        rtiles.astype(BF16).transpose(1, 0, 2).reshape(128, 32 * 512))
    tb_row = np.asarray(tb, np.float32).astype(BF16).reshape(1, S)
    ones_row = np.ones((1, 128), BF16)
    csum_row = np.cumsum(tw).astype(BF16).reshape(1, S)

    h2v = h2.reshape(B, S, E)
    yv = y.reshape(B, S, E)
    in_maps2 = []
    for c in range(NCORES):
        e0 = c * EPC
        h2sl = np.ascontiguousarray(
            h2v[:, :, e0:e0 + EPC].transpose(1, 0, 2).reshape(32, 128, BE)
            .transpose(1, 0, 2).reshape(128, 32 * BE))
        ysl = np.ascontiguousarray(
            yv[:, :, e0:e0 + EPC].transpose(0, 2, 1).reshape(BE, S))
        g = tn_g[e0:e0 + EPC]
        g_safe = np.where(g == 0, 1.0, g)
        in_maps2.append({
            "h2t": h2sl, "rt": rtiles_bf, "yt": ysl,
            "tb": tb_row, "ones": (1.0 / g_safe).astype(BF16).reshape(1, 128),
            "bg": (tn_b[e0:e0 + EPC] / g_safe).astype(BF16).reshape(1, 128),
            "csum": csum_row,
            "gcol": g.astype(np.float32).reshape(128, 1)})
    r2 = _run(p2, in_maps2, trace)
    if trace:
        LAST_TIMINGS["phase2_ns"] = r2.exec_time_ns

    out = np.empty((B, S, E), np.float32)
    for c in range(NCORES):
        e0 = c * EPC
        o = np.asarray(r2.results[c]["out"], np.float32).reshape(B, EPC, S)
        out[:, :, e0:e0 + EPC] = o.transpose(0, 2, 1)
    return out


# revision 19
# speedup vs baseline: 1.2282x; 1.0115x over previous
"""MixerBlock TRN2 kernel: B=2, S=4096, E=1024, DF=4096 on 8 NeuronCores.

Strategy (two SPMD launches):
  Phase 1 (shard B*S=8192 rows -> 1024 rows/core):
    h   = LN(x)            (cn affine folded into W1/b1 host-side)
    a   = silu(h @ W1g + b1')        -> kept transposed aT[df, tok]
    y   = x + aT.T @ W2 + b2
    h2  = LN(y)*tn_g + tn_b          (bf16)
    outputs y (f32), h2 (bf16)
  Phase 2 (shard E=1024 -> 128 channels/core; rows (b,e) = 256/core):
    out[be, s] = sum_t h2T[t, be] * M[t, s] + tb[s] + y[be, s]
    The Toeplitz matrix M[t,s] = tw[s-t] (s>=t) is diagonal-constant, so a
    [128t x 512s] tile depends only on (512*sb - 128*t): 32 distinct tiles,
    prebuilt host-side from tw (4 MB bf16), used as the moving operand.
"""

import os
import sys

sys.path.insert(0, "/opt/trn_rl_repo")
sys.path.insert(0, "/opt/trn_rl_repo/concourse")

import numpy as np
import ml_dtypes

import concourse.bass as bass
import concourse.bacc as bacc
import concourse.mybir as mybir
from concourse import tile
from concourse import bass_utils
from concourse.bass_interp import get_hw_module

dt = mybir.dt
AF = mybir.ActivationFunctionType
AX = mybir.AxisListType
BF16 = ml_dtypes.bfloat16

B, S, E = 2, 4096, 1024
DF = 4 * E
EPS = 1e-5
NCORES = 8
RPC = (B * S) // NCORES      # 1024 rows per core (phase 1)
EPC = E // NCORES            # 128 channels per core (phase 2)
BE = B * EPC                 # 256 (b,e) rows per core (phase 2)

LAST_TIMINGS = {}

# --------------------------------------------------------------------------
# phase 1 program
# --------------------------------------------------------------------------


def build_phase1():
    nc = bacc.Bacc("TRN2", target_bir_lowering=False, debug=False,
                   enable_asserts=False, num_devices=NCORES)
    x_d = nc.dram_tensor("x", [RPC, E], dt.float32, kind="ExternalInput").ap()
    xb_d = nc.dram_tensor("xb", [RPC, E], dt.float32, kind="ExternalInput").ap()
    w1_d = nc.dram_tensor("w1", [E, DF], dt.bfloat16, kind="ExternalInput").ap()
    w2_d = nc.dram_tensor("w2", [DF, E], dt.bfloat16, kind="ExternalInput").ap()
    b1_d = nc.dram_tensor("b1", [128, 32], dt.float32, kind="ExternalInput").ap()
    id_d = nc.dram_tensor("ident", [128, 128], dt.bfloat16, kind="ExternalInput").ap()
    y_d = nc.dram_tensor("y", [RPC, E], dt.float32, kind="ExternalOutput").ap()
    st_d = nc.dram_tensor("st", [RPC, 2], dt.float32, kind="ExternalOutput").ap()

    NT = 4          # token tiles per block (block = 512 tokens)
    NBLK = RPC // (128 * NT)   # 2 blocks

    from contextlib import ExitStack
    with tile.TileContext(nc) as tc, ExitStack() as es:
        pool = lambda **kw: es.enter_context(tc.tile_pool(**kw))
        constp = pool(name="const", bufs=1)
        w1p = pool(name="w1p", bufs=8)
        xp = pool(name="xp", bufs=4)
        xrp = pool(name="xrp", bufs=5)
        statp = pool(name="stat", bufs=24)
        hbfp = pool(name="hbf", bufs=2)
        htp = pool(name="htp", bufs=17)
        atp = pool(name="atp", bufs=33)
        w2p = pool(name="w2p", bufs=4)
        yp = pool(name="yp", bufs=4)
        mps = pool(name="mps", bufs=8, space="PSUM")
        if True:
            # warmup junk tile (no DMA needed) + consts
            junk = constp.tile([128, 512], dt.bfloat16, tag="junk")
            nc.gpsimd.memset(junk[:, :], 0.25)
            id_sb = constp.tile([128, 128], dt.bfloat16, tag="ident")
            nc.sync.dma_start(out=id_sb[:, :], in_=id_d[:, :])
            eps_sb = constp.tile([128, 1], dt.float32, tag="eps")
            nc.gpsimd.memset(eps_sb[:, :], EPS)
            # HAM warmup: dense dummy matmuls while first x tiles load
            wps = mps.tile([128, 512], dt.float32, tag="mp", name="warm")
            for i in range(20):
                nc.tensor.matmul(wps[:, :], junk[:, 0:128], junk[:, :],
                                 start=(i == 0), stop=(i == 19))

            def ln_stats(srct):
                """returns mv [128,2] = (mean, rstd) of rows of srct."""
                stats = statp.tile([128, 2, 6], dt.float32, tag="bst")
                for i in range(2):
                    nc.vector.bn_stats(stats[:, i, :],
                                       srct[:, i * 512:(i + 1) * 512])
                mv = statp.tile([128, 2], dt.float32, tag="mv")
                nc.vector.bn_aggr(mv[:, :], stats[:, :, :])
                nc.scalar.activation(mv[:, 1:2], mv[:, 1:2], AF.Sqrt,
                                     scale=1.0, bias=eps_sb[:, :])
                nc.vector.reciprocal(mv[:, 1:2], mv[:, 1:2])
                return mv

            # ---- LN1 + transpose for ALL tokens up front ----
            hT = [[None] * 8 for _ in range(NBLK)]
            for blk in range(NBLK):
                row0 = blk * 128 * NT
                for tt in range(NT):
                    xt = xp.tile([128, E], dt.float32, tag="xt")
                    nc.sync.dma_start(
                        out=xt[:, :],
                        in_=x_d[row0 + tt * 128: row0 + (tt + 1) * 128, :])
                    mv = ln_stats(xt)
                    hb = hbfp.tile([128, E], dt.bfloat16, tag="hb")
                    nc.vector.tensor_scalar(hb[:, :], xt[:, :],
                                            mv[:, 0:1], mv[:, 1:2],
                                            op0=mybir.AluOpType.subtract,
                                            op1=mybir.AluOpType.mult)
                    for e in range(8):
                        pt = mps.tile([128, 128], dt.bfloat16, tag="mp",
                                      name=f"tp{blk}_{tt}_{e}")
                        nc.tensor.transpose(
                            pt[:, :], hb[:, e * 128:(e + 1) * 128], id_sb[:, :])
                        if hT[blk][e] is None:
                            hT[blk][e] = htp.tile([128, 512], dt.bfloat16,
                                                  tag="ht", name=f"ht{blk}_{e}")
                        nc.vector.tensor_copy(
                            hT[blk][e][:, tt * 128:(tt + 1) * 128], pt[:, :])

            # ---- weights (after x in DMA program order) ----
            w1_sb = []
            for i in range(8):
                t = w1p.tile([128, DF], dt.bfloat16, tag="w1sb")
                nc.sync.dma_start(out=t[:, :], in_=w1_d[i * 128:(i + 1) * 128, :])
                w1_sb.append(t)
            b1_sb = constp.tile([128, 32], dt.float32, tag="b1")
            nc.sync.dma_start(out=b1_sb[:, :], in_=b1_d[:, :])

            for blk in range(NBLK):
                row0 = blk * 128 * NT
                # ---- mm1 + silu -> aT[df][df 128, tok 512] (bf16) ----
                aT = []
                for df in range(32):
                    ps = mps.tile([128, 512], dt.float32, tag="mp",
                                  name=f"m1_{blk}_{df}")
                    for e in range(8):
                        nc.tensor.matmul(
                            ps[:, :],
                            w1_sb[e][:, df * 128:(df + 1) * 128],
                            hT[blk][e][:, :],
                            start=(e == 0), stop=(e == 7))
                    at = atp.tile([128, 512], dt.bfloat16, tag="at")
                    nc.scalar.activation(at[:, :], ps[:, :], AF.Silu,
                                         bias=b1_sb[:, df:df + 1])
                    aT.append(at)
                # ---- prefetch residual (x + b2) rows for this block ----
                xr_t = []
                for tt in range(NT):
                    xr = xrp.tile([128, E], dt.float32, tag="xr",
                                  name=f"xr{blk}_{tt}")
                    nc.sync.dma_start(
                        out=xr[:, :],
                        in_=xb_d[row0 + tt * 128: row0 + (tt + 1) * 128, :])
                    xr_t.append(xr)
                # ---- mm2: df-outer, stream full W2 rows; 8 psum banks ----
                pss = [mps.tile([128, 512], dt.float32, tag="mp",
                                name=f"m2_{blk}_{i}") for i in range(8)]
                for df in range(32):
                    w2t = w2p.tile([128, E], dt.bfloat16, tag="w2t")
                    nc.sync.dma_start(
                        out=w2t[:, :], in_=w2_d[df * 128:(df + 1) * 128, :])
                    for tt in range(NT):
                        for eb in range(2):
                            nc.tensor.matmul(
                                pss[tt * 2 + eb][:, :],
                                aT[df][:, tt * 128:(tt + 1) * 128],
                                w2t[:, eb * 512:(eb + 1) * 512],
                                start=(df == 0), stop=(df == 31))
                # ---- drain: y = psum + (x+b2); write y; LN2 stats out ----
                for tt in range(NT):
                    y_t = yp.tile([128, E], dt.float32, tag="yt",
                                  name=f"yt{blk}_{tt}")
                    for eb in range(2):
                        nc.vector.tensor_add(
                            y_t[:, eb * 512:(eb + 1) * 512],
                            pss[tt * 2 + eb][:, :],
                            xr_t[tt][:, eb * 512:(eb + 1) * 512])
                    nc.gpsimd.dma_start(
                        out=y_d[row0 + tt * 128: row0 + (tt + 1) * 128, :],
                        in_=y_t[:, :])
                    mv2 = ln_stats(y_t)
                    nc.gpsimd.dma_start(
                        out=st_d[row0 + tt * 128: row0 + (tt + 1) * 128, :],
                        in_=mv2[:, :])
    nc.compile()
    nc.m = get_hw_module(nc.m)
    return nc


# --------------------------------------------------------------------------
# phase 2 program
# --------------------------------------------------------------------------


def build_phase2():
    nc = bacc.Bacc("TRN2", target_bir_lowering=False, debug=False,
                   enable_asserts=False, num_devices=NCORES)
    # packed layouts: y2_d[p, t*BE + be] = yT[t*128+p, be]  (bf16)
    #                 r_d[p, d*512 + j] = R[d][p, j]
    #                 stp_d[p, 2*t + k] = (mean, rstd) of token t*128+p
    y2_d = nc.dram_tensor("y2", [128, 32 * BE], dt.bfloat16, kind="ExternalInput").ap()
    r_d = nc.dram_tensor("rt", [128, 32 * 512], dt.bfloat16, kind="ExternalInput").ap()
    stp_d = nc.dram_tensor("stp", [128, 128], dt.float32, kind="ExternalInput").ap()
    yt_d = nc.dram_tensor("yt", [BE, S], dt.float32, kind="ExternalInput").ap()
    tb_d = nc.dram_tensor("tb", [1, S], dt.bfloat16, kind="ExternalInput").ap()
    ones_d = nc.dram_tensor("ones", [1, 128], dt.bfloat16, kind="ExternalInput").ap()
    bg_d = nc.dram_tensor("bg", [1, 128], dt.bfloat16, kind="ExternalInput").ap()
    csum_d = nc.dram_tensor("csum", [1, S], dt.bfloat16, kind="ExternalInput").ap()
    gcol_d = nc.dram_tensor("gcol", [128, 1], dt.float32, kind="ExternalInput").ap()
    out_d = nc.dram_tensor("out", [BE, S], dt.float32, kind="ExternalOutput").ap()

    from contextlib import ExitStack
    with tile.TileContext(nc) as tc, ExitStack() as es:
        pool = lambda **kw: es.enter_context(tc.tile_pool(**kw))
        y2p = pool(name="y2", bufs=4)
        hsp = pool(name="hs", bufs=32)
        rtp = pool(name="rt", bufs=8)
        constp = pool(name="const", bufs=1)
        yinp = pool(name="yin", bufs=6)
        outp = pool(name="outp", bufs=6)
        psp = pool(name="ps", bufs=8, space="PSUM")
        if True:
            # warmup while the first chunks load
            junk = constp.tile([128, 512], dt.bfloat16, tag="junk")
            nc.gpsimd.memset(junk[:, :], 0.25)
            wps = psp.tile([128, 512], dt.float32, tag="ps", name="warm")
            for i in range(24):
                nc.tensor.matmul(wps[:, :], junk[:, 0:128], junk[:, :],
                                 start=(i == 0), stop=(i == 23))

            # chunked loads in consumption order; chunk 0 split 4-way so it
            # lands fast (parallel DMA queues)
            y2_t = [None] * 4   # [128, 2048] each (8 t-tiles)
            rt_t = [None] * 8   # [128, 2048] each (4 d-tiles)

            def load_rt(c, nsplit=1):
                rt_t[c] = rtp.tile([128, 2048], dt.bfloat16, tag="rt",
                                   name=f"rt{c}")
                w = 2048 // nsplit
                for k in range(nsplit):
                    nc.sync.dma_start(
                        out=rt_t[c][:, k * w:(k + 1) * w],
                        in_=r_d[:, c * 2048 + k * w: c * 2048 + (k + 1) * w])

            def load_y2(c, nsplit=1):
                y2_t[c] = y2p.tile([128, 2048], dt.bfloat16, tag="y2",
                                   name=f"y2{c}")
                w = 2048 // nsplit
                for k in range(nsplit):
                    nc.sync.dma_start(
                        out=y2_t[c][:, k * w:(k + 1) * w],
                        in_=y2_d[:, c * 2048 + k * w: c * 2048 + (k + 1) * w])

            load_rt(0, nsplit=4)
            load_y2(0, nsplit=4)
            stp_sb = constp.tile([128, 128], dt.float32, tag="stp")
            nc.sync.dma_start(out=stp_sb[:, :], in_=stp_d[:, :])
            tb_sb = constp.tile([1, S], dt.bfloat16, tag="tb")
            nc.sync.dma_start(out=tb_sb[:, :], in_=tb_d[:, :])
            ones_sb = constp.tile([1, 128], dt.bfloat16, tag="ones")
            nc.sync.dma_start(out=ones_sb[:, :], in_=ones_d[:, :])
            bg_sb = constp.tile([1, 128], dt.bfloat16, tag="bg")
            nc.sync.dma_start(out=bg_sb[:, :], in_=bg_d[:, :])
            csum_sb = constp.tile([1, S], dt.bfloat16, tag="csum")
            nc.sync.dma_start(out=csum_sb[:, :], in_=csum_d[:, :])
            gcol_sb = constp.tile([128, 1], dt.float32, tag="gcol")
            nc.sync.dma_start(out=gcol_sb[:, :], in_=gcol_d[:, :])
            order = [("rt", 1), ("rt", 2), ("y2", 1), ("rt", 3), ("rt", 4),
                     ("y2", 2), ("rt", 5), ("rt", 6), ("y2", 3), ("rt", 7)]
            for kind, c in order:
                if kind == "rt":
                    load_rt(c, nsplit=2)
                else:
                    load_y2(c, nsplit=2)

            # normalize: hs[t][:, b*128:(b+1)*128] = (y2 - mean_bt) * rstd_bt
            # stp[p, 4t+2b+k] = (mean, rstd) of row (b, t*128+p)
            hs = [None] * 32
            for t in range(32):
                hs[t] = hsp.tile([128, BE], dt.bfloat16, tag="hs",
                                 name=f"hs{t}")
                for b in range(2):
                    c0 = 4 * t + 2 * b
                    nc.vector.tensor_scalar(
                        hs[t][:, b * 128:(b + 1) * 128],
                        y2_t[t // 8][:, (t % 8) * BE + b * 128:
                                     (t % 8) * BE + (b + 1) * 128],
                        stp_sb[:, c0:c0 + 1], stp_sb[:, c0 + 1:c0 + 2],
                        op0=mybir.AluOpType.subtract, op1=mybir.AluOpType.mult)

            def rt_ap(d):
                return rt_t[d // 4][:, (d % 4) * 512:(d % 4) * 512 + 512]

            for sb in range(8):
                for be in range(2):
                    ps = psp.tile([128, 512], dt.float32, tag="ps",
                                  name=f"ps{sb}_{be}")
                    for t in range(4 * sb + 4):
                        d = 4 * sb - t + 3
                        nc.tensor.matmul(
                            ps[:, :],
                            hs[t][:, be * 128:(be + 1) * 128], rt_ap(d),
                            start=(t == 0), stop=False)
                    nc.tensor.matmul(
                        ps[:, :], ones_sb[:, :],
                        tb_sb[:, sb * 512:(sb + 1) * 512],
                        start=False, stop=False)
                    nc.tensor.matmul(
                        ps[:, :], bg_sb[:, :],
                        csum_sb[:, sb * 512:(sb + 1) * 512],
                        start=False, stop=True)
                    yin = yinp.tile([128, 512], dt.float32, tag="yin")
                    nc.sync.dma_start(
                        out=yin[:, :],
                        in_=yt_d[be * 128:(be + 1) * 128,
                                 sb * 512:(sb + 1) * 512])
                    ot = outp.tile([128, 512], dt.float32, tag="ot")
                    nc.vector.scalar_tensor_tensor(
                        ot[:, :], ps[:, :], gcol_sb[:, 0:1], yin[:, :],
                        op0=mybir.AluOpType.mult, op1=mybir.AluOpType.add)
                    nc.gpsimd.dma_start(
                        out=out_d[be * 128:(be + 1) * 128,
                                  sb * 512:(sb + 1) * 512],
                        in_=ot[:, :])
    nc.compile()
    nc.m = get_hw_module(nc.m)
    return nc


def _install_ntff_hook():
    """The agent image's antenv lacks axon_hooks; synthesize it so
    run_bass_kernel_spmd(trace=True) can capture NTFF profiles."""
    import types
    import antenv

    if "antenv.axon_hooks" in sys.modules:
        return
    mod = types.ModuleType("antenv.axon_hooks")
    state = {"h": None}
    mod.set_axon_ntff_profile_hook = lambda h: state.__setitem__("h", h)
    mod.get_axon_ntff_profile_hook = lambda: state["h"]
    sys.modules["antenv.axon_hooks"] = mod
    antenv.axon_hooks = mod
    from trn_agent_boot.trn_boot import _ntff_profile_via_ctypes

    mod.set_axon_ntff_profile_hook(
        _ntff_profile_via_ctypes("/opt/axon/libaxon_pjrt.so"))
    bass_utils.upload_artifacts = lambda tmpdir: tmpdir


_P1 = None
_P2 = None


def _programs():
    global _P1, _P2
    if _P1 is None:
        _P1 = build_phase1()
    if _P2 is None:
        _P2 = build_phase2()
    return _P1, _P2


def _run(nc, in_maps, trace):
    if trace:
        try:
            _install_ntff_hook()
        except Exception as e:
            print(f"ntff hook install failed: {e}", file=sys.stderr)
            trace = False
    res = bass_utils.run_bass_kernel_spmd(
        nc, in_maps, core_ids=list(range(NCORES)), trace=trace)
    return res


def kernel(x, cn_g, cn_b, W1, b1, W2, b2, tn_g, tn_b, tw, tb):
    trace = os.environ.get("MIXER_TRACE", "0") == "1"
    x = np.asarray(x, np.float32)
    p1, p2 = _programs()

    # ---- host prep (inputs only) ----
    W1 = np.asarray(W1, np.float32)
    W2 = np.asarray(W2, np.float32)
    cn_g = np.asarray(cn_g, np.float32)
    cn_b = np.asarray(cn_b, np.float32)
    w1g = (cn_g[:, None] * W1).astype(BF16)
    b1f = (np.asarray(b1, np.float32) + cn_b @ W1).astype(np.float32)
    b1_t = np.ascontiguousarray(b1f.reshape(32, 128).T)          # [128, 32]
    w2bf = W2.astype(BF16)
    xbf = (x + np.asarray(b2, np.float32)).reshape(B * S, E)     # x + b2
    ident = np.eye(128, dtype=BF16)
    tn_g = np.asarray(tn_g, np.float32)
    tn_b = np.asarray(tn_b, np.float32)

    xf = x.reshape(B * S, E)
    in_maps1 = []
    for c in range(NCORES):
        in_maps1.append({
            "x": np.ascontiguousarray(xf[c * RPC:(c + 1) * RPC]),
            "xb": np.ascontiguousarray(xbf[c * RPC:(c + 1) * RPC]),
            "w1": w1g, "w2": w2bf, "b1": b1_t, "ident": ident,
        })
    r1 = _run(p1, in_maps1, trace)
    if trace:
        LAST_TIMINGS["phase1_ns"] = r1.exec_time_ns
    y = np.concatenate([np.asarray(r1.results[c]["y"], np.float32)
                        for c in range(NCORES)], axis=0)
    st = np.concatenate([np.asarray(r1.results[c]["st"], np.float32)
                         for c in range(NCORES)], axis=0)       # [B*S, 2]

    # ---- phase 2 host glue ----
    tw = np.asarray(tw, np.float32)
    pad = np.zeros(512 + S + 512, np.float32)
    pad[512:512 + S] = tw
    # R[d][i, j] = tw_ext[(d-3)*128 + j - i]
    win = np.lib.stride_tricks.sliding_window_view(pad, 512)   # win[k] = pad[k:k+512]
    rtiles = np.empty((32, 128, 512), np.float32)
    ii = np.arange(128)
    for d in range(32):
        rtiles[d] = win[512 + (d - 3) * 128 - ii]
    rtiles_bf = np.ascontiguousarray(
        rtiles.astype(BF16).transpose(1, 0, 2).reshape(128, 32 * 512))
    tb_row = np.asarray(tb, np.float32).astype(BF16).reshape(1, S)
    csum_row = np.cumsum(tw).astype(BF16).reshape(1, S)

    # per-(b,token) LN2 stats packed [128, 128]: stp[p, 4t+2b+k] = stv[b, t*128+p, k]
    stv = st.reshape(B, S, 2)
    stp = np.ascontiguousarray(
        stv.reshape(2, 32, 128, 2).transpose(2, 1, 0, 3).reshape(128, 128))
    yv = y.reshape(B, S, E)
    in_maps2 = []
    for c in range(NCORES):
        e0 = c * EPC
        ysl_bt = yv[:, :, e0:e0 + EPC]
        y2sl = np.ascontiguousarray(
            ysl_bt.transpose(1, 0, 2).astype(BF16).reshape(32, 128, BE)
            .transpose(1, 0, 2).reshape(128, 32 * BE))
        ysl = np.ascontiguousarray(
            ysl_bt.transpose(0, 2, 1).reshape(BE, S))
        g = tn_g[e0:e0 + EPC]
        g_safe = np.where(g == 0, 1.0, g)
        in_maps2.append({
            "y2": y2sl, "rt": rtiles_bf, "yt": ysl,
            "stp": stp,
            "tb": tb_row, "ones": (1.0 / g_safe).astype(BF16).reshape(1, 128),
            "bg": (tn_b[e0:e0 + EPC] / g_safe).astype(BF16).reshape(1, 128),
            "csum": csum_row,
            "gcol": g.astype(np.float32).reshape(128, 1)})
    r2 = _run(p2, in_maps2, trace)
    if trace:
        LAST_TIMINGS["phase2_ns"] = r2.exec_time_ns

    out = np.empty((B, S, E), np.float32)
    for c in range(NCORES):
        e0 = c * EPC
        o = np.asarray(r2.results[c]["out"], np.float32).reshape(B, EPC, S)
        out[:, :, e0:e0 + EPC] = o.transpose(0, 2, 1)
    return out


# revision 21
# speedup vs baseline: 1.2645x; 1.0296x over previous
"""MixerBlock TRN2 kernel: B=2, S=4096, E=1024, DF=4096 on 8 NeuronCores.

Strategy (two SPMD launches):
  Phase 1 (shard B*S=8192 rows -> 1024 rows/core):
    h   = LN(x)            (cn affine folded into W1/b1 host-side)
    a   = silu(h @ W1g + b1')        -> kept transposed aT[df, tok]
    y   = x + aT.T @ W2 + b2
    h2  = LN(y)*tn_g + tn_b          (bf16)
    outputs y (f32), h2 (bf16)
  Phase 2 (shard E=1024 -> 128 channels/core; rows (b,e) = 256/core):
    out[be, s] = sum_t h2T[t, be] * M[t, s] + tb[s] + y[be, s]
    The Toeplitz matrix M[t,s] = tw[s-t] (s>=t) is diagonal-constant, so a
    [128t x 512s] tile depends only on (512*sb - 128*t): 32 distinct tiles,
    prebuilt host-side from tw (4 MB bf16), used as the moving operand.
"""

import os
import sys

sys.path.insert(0, "/opt/trn_rl_repo")
sys.path.insert(0, "/opt/trn_rl_repo/concourse")

import numpy as np
import ml_dtypes

import concourse.bass as bass
import concourse.bacc as bacc
import concourse.mybir as mybir
from concourse import tile
from concourse import bass_utils
from concourse.bass_interp import get_hw_module

dt = mybir.dt
AF = mybir.ActivationFunctionType
AX = mybir.AxisListType
BF16 = ml_dtypes.bfloat16

B, S, E = 2, 4096, 1024
DF = 4 * E
EPS = 1e-5
NCORES = 8
RPC = (B * S) // NCORES      # 1024 rows per core (phase 1)
EPC = E // NCORES            # 128 channels per core (phase 2)
BE = B * EPC                 # 256 (b,e) rows per core (phase 2)

LAST_TIMINGS = {}

# --------------------------------------------------------------------------
# phase 1 program
# --------------------------------------------------------------------------


def build_phase1():
    nc = bacc.Bacc("TRN2", target_bir_lowering=False, debug=False,
                   enable_asserts=False, num_devices=NCORES)
    x_d = nc.dram_tensor("x", [RPC, E], dt.float32, kind="ExternalInput").ap()
    xb_d = nc.dram_tensor("xb", [RPC, E], dt.float32, kind="ExternalInput").ap()
    w1_d = nc.dram_tensor("w1", [E, DF], dt.bfloat16, kind="ExternalInput").ap()
    w2_d = nc.dram_tensor("w2", [DF, E], dt.bfloat16, kind="ExternalInput").ap()
    b1_d = nc.dram_tensor("b1", [128, 32], dt.float32, kind="ExternalInput").ap()
    id_d = nc.dram_tensor("ident", [128, 128], dt.bfloat16, kind="ExternalInput").ap()
    y_d = nc.dram_tensor("y", [RPC, E], dt.float32, kind="ExternalOutput").ap()
    st_d = nc.dram_tensor("st", [RPC, 2], dt.float32, kind="ExternalOutput").ap()

    NT = 4          # token tiles per block (block = 512 tokens)
    NBLK = RPC // (128 * NT)   # 2 blocks

    from contextlib import ExitStack
    with tile.TileContext(nc) as tc, ExitStack() as es:
        pool = lambda **kw: es.enter_context(tc.tile_pool(**kw))
        constp = pool(name="const", bufs=1)
        w1p = pool(name="w1p", bufs=8)
        xp = pool(name="xp", bufs=4)
        xrp = pool(name="xrp", bufs=5)
        statp = pool(name="stat", bufs=24)
        hbfp = pool(name="hbf", bufs=2)
        htp = pool(name="htp", bufs=17)
        atp = pool(name="atp", bufs=33)
        w2p = pool(name="w2p", bufs=4)
        yp = pool(name="yp", bufs=4)
        mps = pool(name="mps", bufs=8, space="PSUM")
        if True:
            # warmup junk tile (no DMA needed) + consts
            junk = constp.tile([128, 512], dt.bfloat16, tag="junk")
            nc.gpsimd.memset(junk[:, :], 0.25)
            id_sb = constp.tile([128, 128], dt.bfloat16, tag="ident")
            nc.sync.dma_start(out=id_sb[:, :], in_=id_d[:, :])
            eps_sb = constp.tile([128, 1], dt.float32, tag="eps")
            nc.gpsimd.memset(eps_sb[:, :], EPS)
            # HAM warmup: dense dummy matmuls while first x tiles load
            wps = mps.tile([128, 512], dt.float32, tag="mp", name="warm")
            for i in range(20):
                nc.tensor.matmul(wps[:, :], junk[:, 0:128], junk[:, :],
                                 start=(i == 0), stop=(i == 19))

            def ln_stats(srct):
                """returns mv [128,2] = (mean, rstd) of rows of srct."""
                stats = statp.tile([128, 2, 6], dt.float32, tag="bst")
                for i in range(2):
                    nc.vector.bn_stats(stats[:, i, :],
                                       srct[:, i * 512:(i + 1) * 512])
                mv = statp.tile([128, 2], dt.float32, tag="mv")
                nc.vector.bn_aggr(mv[:, :], stats[:, :, :])
                nc.scalar.activation(mv[:, 1:2], mv[:, 1:2], AF.Sqrt,
                                     scale=1.0, bias=eps_sb[:, :])
                nc.vector.reciprocal(mv[:, 1:2], mv[:, 1:2])
                return mv

            hT = [[None] * 8 for _ in range(NBLK)]

            def ln_transpose_tile(blk, tt, bridge):
                row0 = blk * 128 * NT
                xt = xp.tile([128, E], dt.float32, tag="xt",
                             name=f"xt{blk}_{tt}")
                nc.sync.dma_start(
                    out=xt[:, :],
                    in_=x_d[row0 + tt * 128: row0 + (tt + 1) * 128, :])
                mv = ln_stats(xt)
                hb = hbfp.tile([128, E], dt.bfloat16, tag="hb",
                               name=f"hb{blk}_{tt}")
                nc.vector.tensor_scalar(hb[:, :], xt[:, :],
                                        mv[:, 0:1], mv[:, 1:2],
                                        op0=mybir.AluOpType.subtract,
                                        op1=mybir.AluOpType.mult)
                for e in range(8):
                    pt = mps.tile([128, 128], dt.bfloat16, tag="mp",
                                  name=f"tp{blk}_{tt}_{e}")
                    nc.tensor.transpose(
                        pt[:, :], hb[:, e * 128:(e + 1) * 128], id_sb[:, :])
                    if hT[blk][e] is None:
                        hT[blk][e] = htp.tile([128, 512], dt.bfloat16,
                                              tag="ht", name=f"ht{blk}_{e}")
                    nc.scalar.copy(
                        hT[blk][e][:, tt * 128:(tt + 1) * 128], pt[:, :])
                if bridge:
                    # keep the PE warm while the next LN chain completes
                    bps = mps.tile([128, 512], dt.float32, tag="mp",
                                   name=f"bridge{blk}_{tt}")
                    for i in range(6):
                        nc.tensor.matmul(bps[:, :], junk[:, 0:128], junk[:, :],
                                         start=(i == 0), stop=(i == 5))

            # blk0 LN+transpose (with warm bridges), weights load behind
            for tt in range(NT):
                ln_transpose_tile(0, tt, bridge=True)

            # ---- weights (after blk0 x in DMA program order) ----
            w1_sb = []
            for i in range(8):
                t = w1p.tile([128, DF], dt.bfloat16, tag="w1sb")
                nc.sync.dma_start(out=t[:, :], in_=w1_d[i * 128:(i + 1) * 128, :])
                w1_sb.append(t)
            b1_sb = constp.tile([128, 32], dt.float32, tag="b1")
            nc.sync.dma_start(out=b1_sb[:, :], in_=b1_d[:, :])

            for blk in range(NBLK):
                row0 = blk * 128 * NT
                # ---- mm1 + silu -> aT[df][df 128, tok 512] (bf16) ----
                aT = []
                for df in range(32):
                    ps = mps.tile([128, 512], dt.float32, tag="mp",
                                  name=f"m1_{blk}_{df}")
                    for e in range(8):
                        nc.tensor.matmul(
                            ps[:, :],
                            w1_sb[e][:, df * 128:(df + 1) * 128],
                            hT[blk][e][:, :],
                            start=(e == 0), stop=(e == 7))
                    at = atp.tile([128, 512], dt.bfloat16, tag="at")
                    nc.scalar.activation(at[:, :], ps[:, :], AF.Silu,
                                         bias=b1_sb[:, df:df + 1])
                    aT.append(at)
                if blk == 0 and NBLK > 1:
                    # blk1 LN runs on DVE during mm1-blk0; transposes queue
                    # behind mm1 on the PE and execute back-to-back
                    for tt in range(NT):
                        ln_transpose_tile(1, tt, bridge=False)
                # ---- prefetch residual (x + b2) rows for this block ----
                xr_t = []
                for tt in range(NT):
                    xr = xrp.tile([128, E], dt.float32, tag="xr",
                                  name=f"xr{blk}_{tt}")
                    nc.sync.dma_start(
                        out=xr[:, :],
                        in_=xb_d[row0 + tt * 128: row0 + (tt + 1) * 128, :])
                    xr_t.append(xr)
                # ---- mm2: df-outer, stream full W2 rows; 8 psum banks ----
                pss = [mps.tile([128, 512], dt.float32, tag="mp",
                                name=f"m2_{blk}_{i}") for i in range(8)]
                for df in range(32):
                    w2t = w2p.tile([128, E], dt.bfloat16, tag="w2t")
                    nc.sync.dma_start(
                        out=w2t[:, :], in_=w2_d[df * 128:(df + 1) * 128, :])
                    for tt in range(NT):
                        for eb in range(2):
                            nc.tensor.matmul(
                                pss[tt * 2 + eb][:, :],
                                aT[df][:, tt * 128:(tt + 1) * 128],
                                w2t[:, eb * 512:(eb + 1) * 512],
                                start=(df == 0), stop=(df == 31))
                # ---- drain: y = psum + (x+b2); write y; LN2 stats out ----
                for tt in range(NT):
                    y_t = yp.tile([128, E], dt.float32, tag="yt",
                                  name=f"yt{blk}_{tt}")
                    for eb in range(2):
                        nc.vector.tensor_add(
                            y_t[:, eb * 512:(eb + 1) * 512],
                            pss[tt * 2 + eb][:, :],
                            xr_t[tt][:, eb * 512:(eb + 1) * 512])
                    nc.gpsimd.dma_start(
                        out=y_d[row0 + tt * 128: row0 + (tt + 1) * 128, :],
                        in_=y_t[:, :])
                    mv2 = ln_stats(y_t)
                    nc.gpsimd.dma_start(
                        out=st_d[row0 + tt * 128: row0 + (tt + 1) * 128, :],
                        in_=mv2[:, :])
    nc.compile()
    nc.m = get_hw_module(nc.m)
    return nc


# --------------------------------------------------------------------------
# phase 2 program
# --------------------------------------------------------------------------


def build_phase2():
    nc = bacc.Bacc("TRN2", target_bir_lowering=False, debug=False,
                   enable_asserts=False, num_devices=NCORES)
    # packed layouts: y2_d[p, t*BE + be] = yT[t*128+p, be]  (bf16)
    #                 r_d[p, d*512 + j] = R[d][p, j]
    #                 stp_d[p, 2*t + k] = (mean, rstd) of token t*128+p
    y2_d = nc.dram_tensor("y2", [128, 32 * BE], dt.bfloat16, kind="ExternalInput").ap()
    r_d = nc.dram_tensor("rt", [128, 32 * 512], dt.bfloat16, kind="ExternalInput").ap()
    stp_d = nc.dram_tensor("stp", [128, 128], dt.float32, kind="ExternalInput").ap()
    yt_d = nc.dram_tensor("yt", [BE, S], dt.float32, kind="ExternalInput").ap()
    tb_d = nc.dram_tensor("tb", [1, S], dt.bfloat16, kind="ExternalInput").ap()
    ones_d = nc.dram_tensor("ones", [1, 128], dt.bfloat16, kind="ExternalInput").ap()
    bg_d = nc.dram_tensor("bg", [1, 128], dt.bfloat16, kind="ExternalInput").ap()
    csum_d = nc.dram_tensor("csum", [1, S], dt.bfloat16, kind="ExternalInput").ap()
    gcol_d = nc.dram_tensor("gcol", [128, 1], dt.float32, kind="ExternalInput").ap()
    out_d = nc.dram_tensor("out", [BE, S], dt.float32, kind="ExternalOutput").ap()

    from contextlib import ExitStack
    with tile.TileContext(nc) as tc, ExitStack() as es:
        pool = lambda **kw: es.enter_context(tc.tile_pool(**kw))
        y2p = pool(name="y2", bufs=4)
        hsp = pool(name="hs", bufs=32)
        rtp = pool(name="rt", bufs=8)
        constp = pool(name="const", bufs=1)
        yinp = pool(name="yin", bufs=6)
        outp = pool(name="outp", bufs=6)
        psp = pool(name="ps", bufs=8, space="PSUM")
        if True:
            # warmup while the first chunks load
            junk = constp.tile([128, 512], dt.bfloat16, tag="junk")
            nc.gpsimd.memset(junk[:, :], 0.25)
            wps = psp.tile([128, 512], dt.float32, tag="ps", name="warm")
            for i in range(24):
                nc.tensor.matmul(wps[:, :], junk[:, 0:128], junk[:, :],
                                 start=(i == 0), stop=(i == 23))

            # chunked loads in consumption order; chunk 0 split 4-way so it
            # lands fast (parallel DMA queues)
            y2_t = [None] * 4   # [128, 2048] each (8 t-tiles)
            rt_t = [None] * 8   # [128, 2048] each (4 d-tiles)

            def load_rt(c, nsplit=1):
                rt_t[c] = rtp.tile([128, 2048], dt.bfloat16, tag="rt",
                                   name=f"rt{c}")
                w = 2048 // nsplit
                for k in range(nsplit):
                    nc.sync.dma_start(
                        out=rt_t[c][:, k * w:(k + 1) * w],
                        in_=r_d[:, c * 2048 + k * w: c * 2048 + (k + 1) * w])

            def load_y2(c, nsplit=1):
                y2_t[c] = y2p.tile([128, 2048], dt.bfloat16, tag="y2",
                                   name=f"y2{c}")
                w = 2048 // nsplit
                for k in range(nsplit):
                    nc.sync.dma_start(
                        out=y2_t[c][:, k * w:(k + 1) * w],
                        in_=y2_d[:, c * 2048 + k * w: c * 2048 + (k + 1) * w])

            load_rt(0, nsplit=4)
            load_y2(0, nsplit=4)
            stp_sb = constp.tile([128, 128], dt.float32, tag="stp")
            nc.sync.dma_start(out=stp_sb[:, :], in_=stp_d[:, :])
            tb_sb = constp.tile([1, S], dt.bfloat16, tag="tb")
            nc.sync.dma_start(out=tb_sb[:, :], in_=tb_d[:, :])
            ones_sb = constp.tile([1, 128], dt.bfloat16, tag="ones")
            nc.sync.dma_start(out=ones_sb[:, :], in_=ones_d[:, :])
            bg_sb = constp.tile([1, 128], dt.bfloat16, tag="bg")
            nc.sync.dma_start(out=bg_sb[:, :], in_=bg_d[:, :])
            csum_sb = constp.tile([1, S], dt.bfloat16, tag="csum")
            nc.sync.dma_start(out=csum_sb[:, :], in_=csum_d[:, :])
            gcol_sb = constp.tile([128, 1], dt.float32, tag="gcol")
            nc.sync.dma_start(out=gcol_sb[:, :], in_=gcol_d[:, :])
            order = [("rt", 1), ("rt", 2), ("y2", 1), ("rt", 3), ("rt", 4),
                     ("y2", 2), ("rt", 5), ("rt", 6), ("y2", 3), ("rt", 7)]
            for kind, c in order:
                if kind == "rt":
                    load_rt(c, nsplit=2)
                else:
                    load_y2(c, nsplit=2)

            # normalize on ACT just-in-time: hs[t] half = y2*rstd + (-mean*rstd)
            # stp[p, 4t+2b+k] = (-mean*rstd, rstd) of row (b, t*128+p)
            hs = [None] * 32

            def make_hs(t):
                hs[t] = hsp.tile([128, BE], dt.bfloat16, tag="hs",
                                 name=f"hs{t}")
                for b in range(2):
                    c0 = 4 * t + 2 * b
                    nc.scalar.activation(
                        hs[t][:, b * 128:(b + 1) * 128],
                        y2_t[t // 8][:, (t % 8) * BE + b * 128:
                                     (t % 8) * BE + (b + 1) * 128],
                        AF.Identity,
                        scale=stp_sb[:, c0 + 1:c0 + 2],
                        bias=stp_sb[:, c0:c0 + 1])

            def rt_ap(d):
                return rt_t[d // 4][:, (d % 4) * 512:(d % 4) * 512 + 512]

            for sb in range(8):
                for t in range(4 * sb, 4 * sb + 4):
                    make_hs(t)
                for be in range(2):
                    ps = psp.tile([128, 512], dt.float32, tag="ps",
                                  name=f"ps{sb}_{be}")
                    for t in range(4 * sb + 4):
                        d = 4 * sb - t + 3
                        nc.tensor.matmul(
                            ps[:, :],
                            hs[t][:, be * 128:(be + 1) * 128], rt_ap(d),
                            start=(t == 0), stop=False)
                    nc.tensor.matmul(
                        ps[:, :], ones_sb[:, :],
                        tb_sb[:, sb * 512:(sb + 1) * 512],
                        start=False, stop=False)
                    nc.tensor.matmul(
                        ps[:, :], bg_sb[:, :],
                        csum_sb[:, sb * 512:(sb + 1) * 512],
                        start=False, stop=True)
                    yin = yinp.tile([128, 512], dt.float32, tag="yin")
                    nc.sync.dma_start(
                        out=yin[:, :],
                        in_=yt_d[be * 128:(be + 1) * 128,
                                 sb * 512:(sb + 1) * 512])
                    ot = outp.tile([128, 512], dt.float32, tag="ot")
                    nc.vector.scalar_tensor_tensor(
                        ot[:, :], ps[:, :], gcol_sb[:, 0:1], yin[:, :],
                        op0=mybir.AluOpType.mult, op1=mybir.AluOpType.add)
                    nc.gpsimd.dma_start(
                        out=out_d[be * 128:(be + 1) * 128,
                                  sb * 512:(sb + 1) * 512],
                        in_=ot[:, :])
    nc.compile()
    nc.m = get_hw_module(nc.m)
    return nc


def _install_ntff_hook():
    """The agent image's antenv lacks axon_hooks; synthesize it so
    run_bass_kernel_spmd(trace=True) can capture NTFF profiles."""
    import types
    import antenv

    if "antenv.axon_hooks" in sys.modules:
        return
    mod = types.ModuleType("antenv.axon_hooks")
    state = {"h": None}
    mod.set_axon_ntff_profile_hook = lambda h: state.__setitem__("h", h)
    mod.get_axon_ntff_profile_hook = lambda: state["h"]
    sys.modules["antenv.axon_hooks"] = mod
    antenv.axon_hooks = mod
    from trn_agent_boot.trn_boot import _ntff_profile_via_ctypes

    mod.set_axon_ntff_profile_hook(
        _ntff_profile_via_ctypes("/opt/axon/libaxon_pjrt.so"))
    bass_utils.upload_artifacts = lambda tmpdir: tmpdir


_P1 = None
_P2 = None


def _programs():
    global _P1, _P2
    if _P1 is None:
        _P1 = build_phase1()
    if _P2 is None:
        _P2 = build_phase2()
    return _P1, _P2


def _run(nc, in_maps, trace):
    if trace:
        try:
            _install_ntff_hook()
        except Exception as e:
            print(f"ntff hook install failed: {e}", file=sys.stderr)
            trace = False
    res = bass_utils.run_bass_kernel_spmd(
        nc, in_maps, core_ids=list(range(NCORES)), trace=trace)
    return res


def kernel(x, cn_g, cn_b, W1, b1, W2, b2, tn_g, tn_b, tw, tb):
    trace = os.environ.get("MIXER_TRACE", "0") == "1"
    x = np.asarray(x, np.float32)
    p1, p2 = _programs()

    # ---- host prep (inputs only) ----
    W1 = np.asarray(W1, np.float32)
    W2 = np.asarray(W2, np.float32)
    cn_g = np.asarray(cn_g, np.float32)
    cn_b = np.asarray(cn_b, np.float32)
    w1g = (cn_g[:, None] * W1).astype(BF16)
    b1f = (np.asarray(b1, np.float32) + cn_b @ W1).astype(np.float32)
    b1_t = np.ascontiguousarray(b1f.reshape(32, 128).T)          # [128, 32]
    w2bf = W2.astype(BF16)
    xbf = (x + np.asarray(b2, np.float32)).reshape(B * S, E)     # x + b2
    ident = np.eye(128, dtype=BF16)
    tn_g = np.asarray(tn_g, np.float32)
    tn_b = np.asarray(tn_b, np.float32)

    xf = x.reshape(B * S, E)
    in_maps1 = []
    for c in range(NCORES):
        in_maps1.append({
            "x": np.ascontiguousarray(xf[c * RPC:(c + 1) * RPC]),
            "xb": np.ascontiguousarray(xbf[c * RPC:(c + 1) * RPC]),
            "w1": w1g, "w2": w2bf, "b1": b1_t, "ident": ident,
        })
    r1 = _run(p1, in_maps1, trace)
    if trace:
        LAST_TIMINGS["phase1_ns"] = r1.exec_time_ns
    y = np.concatenate([np.asarray(r1.results[c]["y"], np.float32)
                        for c in range(NCORES)], axis=0)
    st = np.concatenate([np.asarray(r1.results[c]["st"], np.float32)
                         for c in range(NCORES)], axis=0)       # [B*S, 2]

    # ---- phase 2 host glue ----
    tw = np.asarray(tw, np.float32)
    pad = np.zeros(512 + S + 512, np.float32)
    pad[512:512 + S] = tw
    # R[d][i, j] = tw_ext[(d-3)*128 + j - i]
    win = np.lib.stride_tricks.sliding_window_view(pad, 512)   # win[k] = pad[k:k+512]
    rtiles = np.empty((32, 128, 512), np.float32)
    ii = np.arange(128)
    for d in range(32):
        rtiles[d] = win[512 + (d - 3) * 128 - ii]
    rtiles_bf = np.ascontiguousarray(
        rtiles.astype(BF16).transpose(1, 0, 2).reshape(128, 32 * 512))
    tb_row = np.asarray(tb, np.float32).astype(BF16).reshape(1, S)
    csum_row = np.cumsum(tw).astype(BF16).reshape(1, S)

    # per-(b,token) LN2 stats packed [128, 128]: stp[p, 4t+2b+k] = stv[b, t*128+p, k]
    stv = st.reshape(B, S, 2)
    stm = np.stack([-stv[..., 0] * stv[..., 1], stv[..., 1]], axis=-1)
    stp = np.ascontiguousarray(
        stm.reshape(2, 32, 128, 2).transpose(2, 1, 0, 3).reshape(128, 128))
    yv = y.reshape(B, S, E)
    in_maps2 = []
    for c in range(NCORES):
        e0 = c * EPC
        ysl_bt = yv[:, :, e0:e0 + EPC]
        y2sl = np.ascontiguousarray(
            ysl_bt.transpose(1, 0, 2).astype(BF16).reshape(32, 128, BE)
            .transpose(1, 0, 2).reshape(128, 32 * BE))
        ysl = np.ascontiguousarray(
            ysl_bt.transpose(0, 2, 1).reshape(BE, S))
        g = tn_g[e0:e0 + EPC]
        g_safe = np.where(g == 0, 1.0, g)
        in_maps2.append({
            "y2": y2sl, "rt": rtiles_bf, "yt": ysl,
            "stp": stp,
            "tb": tb_row, "ones": (1.0 / g_safe).astype(BF16).reshape(1, 128),
            "bg": (tn_b[e0:e0 + EPC] / g_safe).astype(BF16).reshape(1, 128),
            "csum": csum_row,
            "gcol": g.astype(np.float32).reshape(128, 1)})
    r2 = _run(p2, in_maps2, trace)
    if trace:
        LAST_TIMINGS["phase2_ns"] = r2.exec_time_ns

    out = np.empty((B, S, E), np.float32)
    for c in range(NCORES):
        e0 = c * EPC
        o = np.asarray(r2.results[c]["out"], np.float32).reshape(B, EPC, S)
        out[:, :, e0:e0 + EPC] = o.transpose(0, 2, 1)
    return out
